# revision 23
# baseline (speedup 1.0000x reference)
"""Trainium2 Bass kernel for per-position grouped-query attention.

Reference computation (B=4, S=4096, HID=2048, H=16, G=4, D=128, KV=512):
    q = x @ Wq + bq ; k = x @ Wk + bk ; v = x @ Wv + bv
    scores[t,h,g] = <q[t,h,:], k[t,g,:]> / sqrt(D)     (same-position only)
    probs = softmax_g(scores)
    o[t,h,:] = sum_g probs[t,h,g] * v[t,g,:]
    y = o @ Wo + bo

Strategy: data-parallel over the 16384 flattened tokens -> 2048 tokens/core
on 8 cores, all weights replicated, no collectives.  The matmuls run as
fp8-e4m3 DoubleRow (2 contraction blocks per instruction, 0.5 cyc/row ->
4x bf16 MAC rate), with compensated splits to stay inside the 2e-2 gate:
  - x is shipped as an fp8 (hi, lo) pair: xl = fp8(x - fp8(x)).
  - Q/K projections: (xh + xl) @ fp8(512 W)  - 2 chain segments each; the
    remaining weight-quantization noise only reaches the output through the
    4-way softmax, measured ~1.1e-2 end to end.
  - V projection: xh@Wvh + xl@Wvh + xh@Wvl  (weights split hi/lo) - ~exact.
  - attention middle on DVE/ACT exactly as before (bf16 staging, ~0.1%).
  - O^T (bf16, via DMA-xbar transpose) is split on-chip into fp8 hi/lo
    (ACT cast + DVE subtract) and o @ Wo runs the same 3-chain compensated
    form.  Scale bookkeeping: weights x512, o x16, exp scale /512^2,
    y copy /8192.
Per tile the PE does 51200 cycles (vs 81920 bf16) -> ~341us across 16
tiles; weight DMA (15 MiB fp8) overlaps the 2-tile prologue like before.
"""

import os
import sys

import numpy as np

sys.path.insert(0, "/opt/trn_rl_repo")

import ml_dtypes  # noqa: E402
from contextlib import ExitStack  # noqa: E402

import concourse.bass as bass  # noqa: E402
import concourse.bacc as bacc  # noqa: E402
import concourse.mybir as mybir  # noqa: E402
import concourse.tile as tile  # noqa: E402
from concourse.bass import ds  # noqa: E402
from concourse.bass_utils import run_bass_kernel_spmd  # noqa: E402
from concourse.masks import make_identity  # noqa: E402

B, S, HID = 4, 4096, 2048
H, G = 16, 4
D = HID // H          # 128
KV = HID * G // H     # 512
NCORES = 8
NTOK = B * S          # 16384
TPC = NTOK // NCORES  # 2048 tokens per core
P = 128
NTT = TPC // P        # 16 token tiles per core
NI = HID // P         # 16 input-feature blocks
NSP = NI // 2         # 8 DoubleRow step-pairs over the contraction
SCALE = 1.0 / float(np.sqrt(D))
WS = 512.0            # weight fp8 scale
OS = 16.0             # o fp8 scale

BF16 = mybir.dt.bfloat16
F32 = mybir.dt.float32
F8 = mybir.dt.float8e4
DR = mybir.MatmulPerfMode.DoubleRow

_cache = {}


def _build_fp8() -> bass.Bass:
    """No-bias fast path: fp8 DoubleRow matmuls with compensated splits."""
    nc = bacc.Bacc("TRN2")
    # xh/xl: host-pretransposed per token tile: row (t*128+p), col (i*128+tok)
    # holds x[t*128+tok, i*128+p]  -> per tile a plain [128, 2048] slice whose
    # block i is the lhsT [feat-in-block, token] for the QKV matmuls.
    xh = nc.dram_tensor("xh", [TPC, HID], F8, kind="ExternalInput")
    xl = nc.dram_tensor("xl", [TPC, HID], F8, kind="ExternalInput")
    wq = nc.dram_tensor("wq", [HID, HID], F8, kind="ExternalInput")
    wk = nc.dram_tensor("wk", [HID, KV], F8, kind="ExternalInput")
    wvh = nc.dram_tensor("wvh", [HID, KV], F8, kind="ExternalInput")
    wvl = nc.dram_tensor("wvl", [HID, KV], F8, kind="ExternalInput")
    woh = nc.dram_tensor("woh", [HID, HID], F8, kind="ExternalInput")
    wol = nc.dram_tensor("wol", [HID, HID], F8, kind="ExternalInput")
    y = nc.dram_tensor("y", [TPC, HID], BF16, kind="ExternalOutput")

    with tile.TileContext(nc) as tc, ExitStack() as ctx:
        w_pool = ctx.enter_context(tc.tile_pool(name="w", bufs=1))
        xh_pool = ctx.enter_context(tc.tile_pool(name="xh", bufs=4))
        xl_pool = ctx.enter_context(tc.tile_pool(name="xl", bufs=4))
        ysb_pool = ctx.enter_context(tc.tile_pool(name="ysb", bufs=2))
        qkv_ps_pool = ctx.enter_context(
            tc.tile_pool(name="qkvps", bufs=4, space="PSUM"))
        y_ps_pool = ctx.enter_context(tc.tile_pool(name="yps", bufs=4, space="PSUM"))
        qsb_pool = ctx.enter_context(tc.tile_pool(name="qsb", bufs=3))
        sm_pool = ctx.enter_context(tc.tile_pool(name="sm", bufs=2))
        wt_pool = ctx.enter_context(tc.tile_pool(name="wt", bufs=1))
        obf_pool = ctx.enter_context(tc.tile_pool(name="obf", bufs=2))
        ot_pool = ctx.enter_context(tc.tile_pool(name="ot", bufs=1))
        oth_pool = ctx.enter_context(tc.tile_pool(name="oth", bufs=4))
        otl_pool = ctx.enter_context(tc.tile_pool(name="otl", bufs=4))

        xh_sb = [None] * NTT
        xl_sb = [None] * NTT
        sm_sb = [None] * NTT
        obf_sb = [None] * NTT
        ot_sb = [None] * NTT
        oth_sb = [None] * NTT
        otl_sb = [None] * NTT

        def load_xh(t):
            xh_sb[t] = xh_pool.tile([P, HID], F8, name="xh", tag="xh")
            nc.sync.dma_start(xh_sb[t][:], xh[t * P:(t + 1) * P, :])

        def load_xl(t):
            xl_sb[t] = xl_pool.tile([P, HID], F8, name="xl", tag="xl")
            nc.sync.dma_start(xl_sb[t][:], xl[t * P:(t + 1) * P, :])

        # DMA emission order is the serial-DMA schedule: weights arrive in
        # the prologue's consumption order (chunked by contraction block) so
        # the DMA-bound startup overlaps the first two tiles' compute.
        xh_sb[0] = xh_pool.tile([P, HID], F8, name="xh", tag="xh")
        nc.sync.dma_start(xh_sb[0][:, 0:2 * P], xh[0:P, 0:2 * P])
        nc.sync.dma_start(xh_sb[0][:, 2 * P:], xh[0:P, 2 * P:])
        load_xh(1)

        wqa = w_pool.tile([P, NI * HID], F8, tag="wqa", name="wqa")
        wka = w_pool.tile([P, NI * KV], F8, tag="wka", name="wka")
        wvha = w_pool.tile([P, NI * KV], F8, tag="wvha", name="wvha")
        wvla = w_pool.tile([P, NI * KV], F8, tag="wvla", name="wvla")
        woha = w_pool.tile([P, NI * HID], F8, tag="woha", name="woha")
        wola = w_pool.tile([P, NI * HID], F8, tag="wola", name="wola")

        def wchunk(dst_all, src, i0, i1, c0, c1, w_):
            # weight blocks i0..i1-1, cols c0:c1, one DMA
            nc.sync.dma_start(
                dst_all[:, i0 * w_:i1 * w_]
                .rearrange("p (i c) -> p i c", c=w_)[:, :, c0:c1],
                src[i0 * P:i1 * P, c0:c1]
                .rearrange("(i p) c -> p i c", p=P),
            )

        # Prologue covers 4 tiles (phases A/B/C below): with fp8 DoubleRow
        # the PE retires weight bytes 4x faster than bf16, so only >=4
        # concurrent reader tiles keep the serial weight DMA stream off the
        # critical path.  2-block chunks match the chains' pair consumption;
        # xl0-3 land mid-stream (xl needed from chain step 8).
        PAIRS = [(2 * j, 2 * j + 2) for j in range(NSP)]
        load_xh(2)
        # phase A weights: wq cols 0:1024
        wchunk(wqa, wq, 0, 2, 0, 1024, HID)
        load_xl(0)
        wchunk(wqa, wq, 2, 4, 0, 1024, HID)
        load_xl(1)
        wchunk(wqa, wq, 4, 6, 0, 1024, HID)
        load_xl(2)
        for i0, i1 in PAIRS[3:]:
            wchunk(wqa, wq, i0, i1, 0, 1024, HID)
        # phase B weights: wq cols 1024:2048
        for i0, i1 in PAIRS:
            wchunk(wqa, wq, i0, i1, 1024, 2048, HID)
        # phase C weights: all of wk (its phase runs first so tile 0's
        # scores start early), then wvh, then wvl (consumed only from chain
        # step 16 of the V chains)
        for i0, i1 in PAIRS:
            wchunk(wka, wk, i0, i1, 0, KV, KV)
        for i0, i1 in PAIRS:
            wchunk(wvha, wvh, i0, i1, 0, KV, KV)
        for i0, i1 in PAIRS:
            wchunk(wvla, wvl, i0, i1, 0, KV, KV)
        # first half of wo-hi rides the phase-C DMA slack (phase C PE time
        # well exceeds its own weight bytes)
        for j in range(4):
            wchunk(woha, woh, 4 * j, 4 * (j + 1), 0, 1024, HID)

        wq_r = wqa[:].rearrange("p (i c) -> p i c", c=HID)
        wk_r = wka[:].rearrange("p (i c) -> p i c", c=KV)
        wvh_r = wvha[:].rearrange("p (i c) -> p i c", c=KV)
        wvl_r = wvla[:].rearrange("p (i c) -> p i c", c=KV)
        woh_r = woha[:].rearrange("p (i c) -> p i c", c=HID)
        wol_r = wola[:].rearrange("p (i c) -> p i c", c=HID)

        def xpair(t, s, lo):
            src = xl_sb[t] if lo else xh_sb[t]
            return src[:, 2 * s * P:(2 * s + 2) * P].rearrange(
                "p (i t2) -> p i t2", t2=P)

        def qkv_steps(t, which, c=0):
            """DoubleRow (lhsT, rhs) step list for one 512-col psum chain."""
            if which == "q":
                segs = [(False, wq_r), (True, wq_r)]
                cs = slice(c * 512, (c + 1) * 512)
            elif which == "k":
                segs = [(False, wk_r), (True, wk_r)]
                cs = slice(0, KV)
            else:  # v
                segs = [(False, wvh_r), (True, wvh_r), (False, wvl_r)]
                cs = slice(0, KV)
            steps = []
            for lo, w_r in segs:
                for s in range(NSP):
                    steps.append((xpair(t, s, lo), w_r[:, 2 * s:2 * s + 2, cs]))
            return steps

        def opair(t, s, lo):
            src = otl_sb[t] if lo else oth_sb[t]
            return src[:, 2 * s * P:(2 * s + 2) * P].rearrange(
                "p (o t2) -> p o t2", t2=P)

        def y_steps(tw, c0, c1):
            # o-half-major over the woh segments (so the chain starts as
            # soon as the first o-split half lands); the wol segment stays
            # last - it is the final arrival in the weight DMA stream
            cs = slice(c0, c1)
            steps = []
            for half in range(2):
                for lo in (False, True):
                    w_r = woh_r
                    for s in range(half * NSP // 2, (half + 1) * NSP // 2):
                        steps.append(
                            (opair(tw, s, lo), w_r[:, 2 * s:2 * s + 2, cs]))
            for s in range(NSP):
                steps.append((opair(tw, s, False), wol_r[:, 2 * s:2 * s + 2, cs]))
            return steps

        def emit_chain_pair(pairs):
            """pairs: list of (psum, steps[, on_done]); interleave
            step-wise, firing on_done(ps) right after a chain's stop so its
            psum copy is emitted as early as possible."""
            n = max(len(p[1]) for p in pairs)
            for s in range(n):
                for p in pairs:
                    ps, st = p[0], p[1]
                    if s < len(st):
                        lhs, rhs = st[s]
                        nc.tensor.matmul(
                            ps[:], lhs, rhs,
                            start=(s == 0), stop=(s == len(st) - 1),
                            perf_mode=DR,
                        )
                        if s == len(st) - 1 and len(p) > 2:
                            p[2](ps)

        HH = H // 2
        HW = HH * D  # 1024, half of the o columns

        def attn_half(t, qsb, ksb, vsb, hf):
            """scores + softmax + weighted sum for 8 heads (DVE + ACT +
            Pool).  Halving lets the o-split and the first y chains start
            one half-attn earlier."""
            if hf == 0:
                sm_sb[t] = (
                    sm_pool.tile([P, H * G], F32, tag="sc", name="sc"),
                    sm_pool.tile([P, H * G], F32, tag="ex", name="ex"),
                    sm_pool.tile([P, H], F32, tag="dn", name="dn"),
                    sm_pool.tile([P, H], F32, tag="rc", name="rc"),
                    sm_pool.tile([P, H * G], F32, tag="pf", name="pf"),
                    sm_pool.tile([P, D], BF16, tag="junk", name="junk"),
                )
                obf_sb[t] = obf_pool.tile([P, HID], BF16, name="obf",
                                          tag="obf")
            sc, ex, dn, rc, pf, junk = sm_sb[t]
            obf = obf_sb[t]
            h0 = hf * HH
            cs = slice(h0 * G, (h0 + HH) * G)

            # raw scores sc[t,(h,g)] = <q_h, k_g>  (fused mult+reduce, DVE)
            for h in range(h0, h0 + HH):
                for g in range(G):
                    nc.vector.scalar_tensor_tensor(
                        junk[:],
                        qsb[:, h * D:(h + 1) * D],
                        1.0,
                        ksb[:, g * D:(g + 1) * D],
                        op0=mybir.AluOpType.mult,
                        op1=mybir.AluOpType.mult,
                        accum_out=sc[:, ds(h * G + g, 1)],
                    )

            # softmax over g; q,k carry x512 each -> exp scale /512^2
            nc.scalar.activation(
                ex[:, cs], sc[:, cs], mybir.ActivationFunctionType.Exp,
                scale=SCALE / (WS * WS))
            nc.vector.reduce_sum(
                dn[:, h0:h0 + HH],
                ex[:, cs].rearrange("p (h g) -> p h g", g=G),
                axis=mybir.AxisListType.X,
            )
            nc.vector.reciprocal(rc[:, h0:h0 + HH], dn[:, h0:h0 + HH])
            # pf = OS * ex / dn  (o shipped as 16*o for the fp8 split)
            nc.vector.scalar_tensor_tensor(
                pf[:, cs].rearrange("p (h g) -> p h g", g=G),
                ex[:, cs].rearrange("p (h g) -> p h g", g=G),
                OS,
                rc[:, h0:h0 + HH].unsqueeze(2).broadcast_to((P, HH, G)),
                op0=mybir.AluOpType.mult, op1=mybir.AluOpType.mult,
            )

            # o[t,(h,d)] = sum_g p[t,(h,g)] * v[t,(g,d)]  - batched per g on
            # the otherwise-idle GpSimd/Pool engine (4 mult + 3 accum ops of
            # [p, HH*D], broadcast views; DVE keeps only the score dots)
            tmp = wt_pool.tile([P, HW], BF16, tag="ta", name="ta")
            tm3 = tmp[:].rearrange("p (h d) -> p h d", d=D)
            ob3 = obf[:, hf * HW:(hf + 1) * HW] \
                .rearrange("p (h d) -> p h d", d=D)
            pfh = pf[:].rearrange("p (h g) -> p h g", g=G)[:, h0:h0 + HH, :]
            vv = lambda g: vsb[:, g * D:(g + 1) * D].unsqueeze(1) \
                .broadcast_to((P, HH, D))
            pp = lambda g: pfh[:, :, g:g + 1].broadcast_to((P, HH, D))
            nc.gpsimd.tensor_tensor(ob3, vv(0), pp(0),
                                    op=mybir.AluOpType.mult)
            for g in range(1, G):
                nc.gpsimd.tensor_tensor(tm3, vv(g), pp(g),
                                        op=mybir.AluOpType.mult)
                nc.gpsimd.tensor_tensor(ob3, ob3, tm3,
                                        op=mybir.AluOpType.add)

        def attn_middle(t, qsb, ksb, vsb):
            attn_half(t, qsb, ksb, vsb, 0)
            attn_half(t, qsb, ksb, vsb, 1)

        def split_half(t, hf):
            # O^T via the DMA xbar (keeps the PE matmul-only):
            # ot[p, o*128+tok] = obf[tok, o*128+p]; then split to fp8 hi/lo.
            if hf == 0:
                ot_sb[t] = ot_pool.tile([P, HID], BF16, name="ot", tag="ot")
                oth_sb[t] = oth_pool.tile([P, HID], F8, name="oth",
                                          tag="oth")
                otl_sb[t] = otl_pool.tile([P, HID], F8, name="otl",
                                          tag="otl")
            ot, oth, otl = ot_sb[t], oth_sb[t], otl_sb[t]
            hs = slice(hf * HW, (hf + 1) * HW)
            nc.sync.dma_start_transpose(
                ot[:, hs].rearrange("p (o t2) -> p o t2", t2=P),
                obf_sb[t][:, hs])
            nc.scalar.copy(oth[:, hs], ot[:, hs])
            nc.gpsimd.tensor_sub(otl[:, hs], ot[:, hs], oth[:, hs])

        def transpose_split_o(t):
            split_half(t, 0)
            split_half(t, 1)
            obf_sb[t] = None
            ot_sb[t] = None

        def copy_qkv(which, c, ps, qsb, ksb, vsb):
            if which == "q":
                nc.scalar.copy(qsb[:, c * 512:(c + 1) * 512], ps[:])
            elif which == "k":
                nc.scalar.copy(ksb[:], ps[:])
            else:
                nc.scalar.mul(vsb[:], ps[:], 1.0 / WS)

        def emit_y_tile(tw):
            if tw == NTT - 1:
                # last tile: sequential chains, final ones narrow, so the
                # post-matmul drain holds only one short copy+DMA
                for c0, c1 in ((0, 512), (512, 1024), (1024, 1536),
                               (1536, 1792), (1792, 2048)):
                    yps = y_ps_pool.tile([P, c1 - c0], F32,
                                         name="yps", tag="yps")
                    emit_chain_pair([(yps, y_steps(tw, c0, c1))])
                    ysb = ysb_pool.tile([P, c1 - c0], BF16,
                                        name="ysb", tag="ysb")
                    nc.scalar.mul(ysb[:], yps[:], 1.0 / (WS * OS))
                    nc.sync.dma_start(
                        y[tw * P:(tw + 1) * P, c0:c1], ysb[:])
                oth_sb[tw] = None
                otl_sb[tw] = None
                return
            for sp in range(2):
                ypa = y_ps_pool.tile([P, 512], F32, name="yps", tag="yps")
                ypb = y_ps_pool.tile([P, 512], F32, name="yps", tag="yps")
                emit_chain_pair([
                    (ypa, y_steps(tw, 2 * sp * 512, (2 * sp + 1) * 512)),
                    (ypb, y_steps(tw, (2 * sp + 1) * 512, (2 * sp + 2) * 512)),
                ])
                for yps, s_ in ((ypa, 2 * sp), (ypb, 2 * sp + 1)):
                    ysb = ysb_pool.tile([P, 512], BF16, name="ysb", tag="ysb")
                    nc.scalar.mul(ysb[:], yps[:], 1.0 / (WS * OS))
                    nc.sync.dma_start(
                        y[tw * P:(tw + 1) * P, s_ * 512:(s_ + 1) * 512],
                        ysb[:])
            oth_sb[tw] = None
            otl_sb[tw] = None

        # ---- prologue: tiles 0-3 in three phases, each running 8 psum
        # chains (all banks) step-major so every weight chunk is consumed
        # by 4 reader tiles as it arrives ----
        NPT = 3
        pro_sb = {}
        for tt in range(NPT):
            pro_sb[tt] = (
                qsb_pool.tile([P, HID], BF16, tag="q", name="q"),
                qsb_pool.tile([P, KV], BF16, tag="k", name="k"),
                qsb_pool.tile([P, KV], BF16, tag="v", name="v"),
            )
        for chains in ([("q", 0), ("q", 1)], [("q", 2), ("q", 3)],
                       [("k", 0)], [("v", 0)]):
            pairs = []
            for ci, (which, c) in enumerate(chains):
                for tt in range(NPT):
                    pool = qkv_ps_pool if ci == 0 else y_ps_pool
                    tagname = "ps" if ci == 0 else "yps"
                    ps = pool.tile([P, 512], F32, name=tagname,
                                   tag=tagname)
                    pairs.append(
                        (ps, qkv_steps(tt, which, c),
                         (lambda w_, c_, t_: lambda psd: copy_qkv(
                             w_, c_, psd, *pro_sb[t_]))(which, c, tt)))
            emit_chain_pair(pairs)
        # attn interleaved per half-tile so each otl subtract sits directly
        # behind its weighted-sum in the Pool queue and the first y chains
        # start after only half an attn.  Only tile 0's split runs here -
        # tiles 1/2 are deferred into the loop so their transpose DMAs queue
        # behind the wo weight stream instead of delaying it.
        for hf in range(2):
            attn_half(0, *pro_sb[0], hf)
            split_half(0, hf)
        obf_sb[0] = None
        ot_sb[0] = None
        for tt in (1, 2):
            attn_middle(tt, *pro_sb[tt])
        load_xh(3)
        load_xl(3)
        load_xh(4)
        load_xl(4)

        # ---- steady state: QKV(t) | Wo(t-3 or t-4); the wo weight stream
        # is emitted at the first loop iteration, after every qkv weight and
        # the early x tiles, so the first y chains never wait on it ----
        for t in range(NPT, NTT + 4):
            if t <= NTT - 3:
                load_xh(t + 2)
                load_xl(t + 2)
            if t == NPT:
                # rest of wo, in Y(0)'s consumption order: its first chain
                # pair tails with wol cols 0:1024, then the second pair
                # opens on woh cols 1024:2048
                for j in range(4):
                    wchunk(wola, wol, 4 * j, 4 * (j + 1), 0, 1024, HID)
                for j in range(4):
                    wchunk(woha, woh, 4 * j, 4 * (j + 1), 1024, 2048, HID)
                for j in range(4):
                    wchunk(wola, wol, 4 * j, 4 * (j + 1), 1024, 2048, HID)
                transpose_split_o(1)
            if t == NPT + 1:
                transpose_split_o(2)

            if t < NTT:
                qsb = qsb_pool.tile([P, HID], BF16, tag="q")
                ksb = qsb_pool.tile([P, KV], BF16, tag="k")
                vsb = qsb_pool.tile([P, KV], BF16, tag="v")

                # 3 pair-interleaved chains; paired chains share the
                # stationary x pair per step (back-to-back reuse)
                for pa, pb in ((("q", 0), ("k", 0)),
                               (("q", 1), ("q", 2)),
                               (("q", 3), ("v", 0))):
                    psa = qkv_ps_pool.tile([P, 512], F32, name="ps", tag="ps")
                    psb = qkv_ps_pool.tile([P, 512], F32, name="ps", tag="ps")
                    emit_chain_pair([
                        (psa, qkv_steps(t, pa[0], pa[1])),
                        (psb, qkv_steps(t, pb[0], pb[1])),
                    ])
                    copy_qkv(pa[0], pa[1], psa, qsb, ksb, vsb)
                    copy_qkv(pb[0], pb[1], psb, qsb, ksb, vsb)

                attn_middle(t, qsb, ksb, vsb)

            # Wo matmuls + y out, lagged 5 tiles behind the QKV stream so
            # the first y chains never wait on the prologue's attn backlog
            # (4 tiles of DVE/Pool work drain at ~8us/iter) nor on the
            # wo weight DMA
            if t - 4 >= 0:
                emit_y_tile(t - 4)

            if NPT <= t < NTT:
                transpose_split_o(t)

    nc.compile()
    return nc


def _build_bias(has_bias: bool = True) -> bass.Bass:
    """Original (slower) path, kept for the biased case."""
    nc = bacc.Bacc("TRN2")
    x = nc.dram_tensor("x", [TPC, HID], BF16, kind="ExternalInput")
    wq = nc.dram_tensor("wq", [HID, HID], BF16, kind="ExternalInput")
    wk = nc.dram_tensor("wk", [HID, KV], BF16, kind="ExternalInput")
    wv = nc.dram_tensor("wv", [HID, KV], BF16, kind="ExternalInput")
    wo = nc.dram_tensor("wo", [HID, HID], BF16, kind="ExternalInput")
    if has_bias:
        bqkv = nc.dram_tensor("bqkv", [1, HID + 2 * KV], F32, kind="ExternalInput")
        bo = nc.dram_tensor("bo", [1, HID], F32, kind="ExternalInput")
    y = nc.dram_tensor("y", [TPC, HID], F32, kind="ExternalOutput")

    with tile.TileContext(nc) as tc, ExitStack() as ctx:
        const_pool = ctx.enter_context(tc.tile_pool(name="const", bufs=1))
        ident = const_pool.tile([P, P], BF16)
        make_identity(nc, ident[:])

        if has_bias:
            bias_qkv = const_pool.tile([P, HID + 2 * KV], F32)
            nc.sync.dma_start(bias_qkv[:], bqkv[0:1, :].broadcast_to((P, HID + 2 * KV)))
            bias_o = const_pool.tile([P, HID], F32)
            nc.sync.dma_start(bias_o[:], bo[0:1, :].broadcast_to((P, HID)))

        # O^T staging for the whole core: [o_block(16) x tokens(2048)] bf16
        ofm_pool = ctx.enter_context(tc.tile_pool(name="ofm", bufs=1))
        ofm = ofm_pool.tile([P, NI * TPC], BF16)

        kv_pool = ctx.enter_context(tc.tile_pool(name="wkv", bufs=1))
        wk_sb = []
        wv_sb = []
        for i in range(NI):
            wk_t = kv_pool.tile([P, KV], BF16, tag=f"wk{i}")
            nc.sync.dma_start(wk_t[:], wk[i * P:(i + 1) * P, :])
            wk_sb.append(wk_t)
            wv_t = kv_pool.tile([P, KV], BF16, tag=f"wv{i}")
            nc.sync.dma_start(wv_t[:], wv[i * P:(i + 1) * P, :])
            wv_sb.append(wv_t)

        pt_pool = ctx.enter_context(tc.tile_pool(name="pt", bufs=2, space="PSUM"))
        mm_pool = ctx.enter_context(tc.tile_pool(name="mm", bufs=3, space="PSUM"))

        # ---------------- Phase A: QKV projections + attention ----------------
        with tc.tile_pool(name="wqp", bufs=1) as wq_pool, \
             tc.tile_pool(name="xt", bufs=3) as xt_pool, \
             tc.tile_pool(name="xfm", bufs=1) as xfm_pool, \
             tc.tile_pool(name="qkv", bufs=1) as qkv_pool, \
             tc.tile_pool(name="attn", bufs=2) as attn_pool, \
             tc.tile_pool(name="oacc", bufs=1) as oacc_pool, \
             tc.tile_pool(name="obf", bufs=1) as obf_pool:
            wq_sb = []
            for i in range(NI):
                wq_t = wq_pool.tile([P, HID], BF16, tag=f"wq{i}")
                nc.sync.dma_start(wq_t[:], wq[i * P:(i + 1) * P, :])
                wq_sb.append(wq_t)

            for t in range(NTT):
                xt = xt_pool.tile([P, HID], BF16)
                nc.sync.dma_start(xt[:], x[t * P:(t + 1) * P, :])

                # transpose X tile to feature-major [i, t] (16 blocks of 128x128)
                xfm = xfm_pool.tile([P, HID], BF16)
                for j in range(4):
                    pt = pt_pool.tile([P, 512], BF16)
                    for k in range(4):
                        blk = 4 * j + k
                        nc.tensor.transpose(
                            pt[:, k * P:(k + 1) * P],
                            xt[:, blk * P:(blk + 1) * P],
                            ident[:],
                        )
                    nc.vector.tensor_copy(xfm[:, j * 512:(j + 1) * 512], pt[:])

                # QKV projections, token-major out: [t(128part), 3072]
                qkv = qkv_pool.tile([P, HID + 2 * KV], F32)
                for s in range(6):
                    ps = mm_pool.tile([P, 512], F32)
                    for i in range(NI):
                        if s < 4:
                            rhs = wq_sb[i][:, s * 512:(s + 1) * 512]
                        elif s == 4:
                            rhs = wk_sb[i][:]
                        else:
                            rhs = wv_sb[i][:]
                        nc.tensor.matmul(
                            ps[:], xfm[:, i * P:(i + 1) * P], rhs,
                            start=(i == 0), stop=(i == NI - 1),
                        )
                    if has_bias:
                        nc.vector.tensor_add(
                            qkv[:, s * 512:(s + 1) * 512], ps[:],
                            bias_qkv[:, s * 512:(s + 1) * 512],
                        )
                    else:
                        nc.vector.tensor_copy(qkv[:, s * 512:(s + 1) * 512], ps[:])

                # scores[t, h, g] = <q_h, k_g> * SCALE   (fused mult+reduce)
                sc = attn_pool.tile([P, H * G], F32, tag="sc")
                junk = attn_pool.tile([P, D], F32, tag="junk")
                for h in range(H):
                    for g in range(G):
                        nc.vector.scalar_tensor_tensor(
                            junk[:],
                            qkv[:, h * D:(h + 1) * D],
                            SCALE,
                            qkv[:, HID + g * D:HID + (g + 1) * D],
                            op0=mybir.AluOpType.mult,
                            op1=mybir.AluOpType.mult,
                            accum_out=sc[:, ds(h * G + g, 1)],
                        )

                # softmax over g (4); denominator folded into final scale
                ex = attn_pool.tile([P, H * G], F32, tag="ex")
                nc.scalar.activation(ex[:], sc[:], mybir.ActivationFunctionType.Exp)
                dn = attn_pool.tile([P, H], F32, tag="dn")
                nc.vector.reduce_sum(
                    dn[:], ex[:].rearrange("p (h g) -> p h g", g=G),
                    axis=mybir.AxisListType.X,
                )
                rc = attn_pool.tile([P, H], F32, tag="rc")
                nc.vector.reciprocal(rc[:], dn[:])

                # o[t, h*D+d] = (sum_g ex[t,h,g] * v[t, g*D+d]) * rc[t,h]
                acc = oacc_pool.tile([P, HID], F32, tag="acc")
                tmp = oacc_pool.tile([P, HID], F32, tag="tmp")
                obf = obf_pool.tile([P, HID], BF16)
                ab = [acc, tmp]
                for h in range(H):
                    hs = ds(h * D, D)
                    nc.vector.tensor_scalar_mul(
                        ab[0][:, hs],
                        qkv[:, HID + KV:HID + KV + D],
                        ex[:, ds(h * G, 1)],
                    )
                    for g in range(1, G):
                        nc.vector.scalar_tensor_tensor(
                            ab[g % 2][:, hs],
                            qkv[:, HID + KV + g * D:HID + KV + (g + 1) * D],
                            ex[:, ds(h * G + g, 1)],
                            ab[(g - 1) % 2][:, hs],
                            op0=mybir.AluOpType.mult,
                            op1=mybir.AluOpType.add,
                        )
                    nc.vector.tensor_scalar_mul(
                        obf[:, hs], ab[(G - 1) % 2][:, hs], rc[:, ds(h, 1)])

                # transpose O tile into ofm [o_block, token]
                for j in range(4):
                    pt = pt_pool.tile([P, 512], BF16)
                    for k in range(4):
                        blk = 4 * j + k
                        nc.tensor.transpose(
                            pt[:, k * P:(k + 1) * P],
                            obf[:, blk * P:(blk + 1) * P],
                            ident[:],
                        )
                    nc.vector.tensor_copy(
                        ofm[:].rearrange("p (o t) -> p o t", t=TPC)
                              [:, 4 * j:4 * j + 4, t * P:(t + 1) * P],
                        pt[:].rearrange("p (o t) -> p o t", t=P),
                    )

        # ---------------- Phase B: output projection ----------------
        with tc.tile_pool(name="wop", bufs=1) as wo_pool, \
             tc.tile_pool(name="yt", bufs=3) as yt_pool:
            wo_sb = []
            for i in range(NI):
                wo_t = wo_pool.tile([P, HID], BF16, tag=f"wo{i}")
                nc.sync.dma_start(wo_t[:], wo[i * P:(i + 1) * P, :])
                wo_sb.append(wo_t)

            for t in range(NTT):
                for s in range(4):
                    ps = mm_pool.tile([P, 512], F32)
                    for o in range(NI):
                        nc.tensor.matmul(
                            ps[:],
                            ofm[:, ds(o * TPC + t * P, P)],
                            wo_sb[o][:, s * 512:(s + 1) * 512],
                            start=(o == 0), stop=(o == NI - 1),
                        )
                    yt = yt_pool.tile([P, 512], F32)
                    if has_bias:
                        nc.vector.tensor_add(
                            yt[:], ps[:], bias_o[:, s * 512:(s + 1) * 512])
                    else:
                        nc.vector.tensor_copy(yt[:], ps[:])
                    nc.sync.dma_start(
                        y[t * P:(t + 1) * P, s * 512:(s + 1) * 512], yt[:])

    nc.compile()
    return nc


def _build(has_bias: bool) -> bass.Bass:
    return _build_bias(True) if has_bias else _build_fp8()


def kernel(hidden_states, Wq, bq, Wk, bk, Wv, bv, Wo, bo, _profile=None):
    has_bias = bool(np.any(bq) or np.any(bk) or np.any(bv) or np.any(bo))
    key = has_bias
    if key not in _cache:
        _cache[key] = _build(has_bias)
    nc = _cache[key]

    x_flat = np.ascontiguousarray(
        np.asarray(hidden_states, dtype=np.float32).reshape(NTOK, HID))

    in_maps = []
    if has_bias:
        bf = ml_dtypes.bfloat16
        xb = x_flat.astype(bf)
        wq_b = np.asarray(Wq, dtype=np.float32).astype(bf)
        wk_b = np.asarray(Wk, dtype=np.float32).astype(bf)
        wv_b = np.asarray(Wv, dtype=np.float32).astype(bf)
        wo_b = np.asarray(Wo, dtype=np.float32).astype(bf)
        for c in range(NCORES):
            m = {
                "x": np.ascontiguousarray(xb[c * TPC:(c + 1) * TPC]),
                "wq": wq_b, "wk": wk_b, "wv": wv_b, "wo": wo_b,
                "bqkv": np.concatenate([
                    np.asarray(bq, np.float32), np.asarray(bk, np.float32),
                    np.asarray(bv, np.float32)]).reshape(1, HID + 2 * KV),
                "bo": np.asarray(bo, np.float32).reshape(1, HID),
            }
            in_maps.append(m)
    else:
        e4 = ml_dtypes.float8_e4m3
        xh8 = x_flat.astype(e4)
        xl8 = (x_flat - xh8.astype(np.float32)).astype(e4)

        def wsplit(W):
            Wf = np.asarray(W, dtype=np.float32) * WS
            hi = Wf.astype(e4)
            lo = (Wf - hi.astype(np.float32)).astype(e4)
            return np.ascontiguousarray(hi), np.ascontiguousarray(lo)

        wq8 = np.ascontiguousarray(
            (np.asarray(Wq, np.float32) * WS).astype(e4))
        wk8 = np.ascontiguousarray(
            (np.asarray(Wk, np.float32) * WS).astype(e4))
        wvh8, wvl8 = wsplit(Wv)
        woh8, wol8 = wsplit(Wo)

        def pret(a):
            # host pre-transpose: row (t*128+p), col (i*128+tok) <- x[(t,tok),(i,p)]
            return np.ascontiguousarray(
                a.reshape(NTT, P, NI, P).transpose(0, 3, 2, 1).reshape(TPC, HID))

        for c in range(NCORES):
            m = {
                "xh": pret(xh8[c * TPC:(c + 1) * TPC]),
                "xl": pret(xl8[c * TPC:(c + 1) * TPC]),
                "wq": wq8, "wk": wk8,
                "wvh": wvh8, "wvl": wvl8,
                "woh": woh8, "wol": wol8,
            }
            in_maps.append(m)

    kwargs = dict(_profile) if _profile else {}
    kwargs.pop("result", None)
    res = run_bass_kernel_spmd(nc, in_maps, list(range(NCORES)), **kwargs)
    out = np.concatenate([r["y"] for r in res.results], axis=0)
    if _profile is not None:
        _profile["result"] = res
    return out.reshape(B, S, HID).astype(np.float32)


# revision 24
# speedup vs baseline: 1.0123x; 1.0123x over previous
"""Trainium2 Bass kernel for per-position grouped-query attention.

Reference computation (B=4, S=4096, HID=2048, H=16, G=4, D=128, KV=512):
    q = x @ Wq + bq ; k = x @ Wk + bk ; v = x @ Wv + bv
    scores[t,h,g] = <q[t,h,:], k[t,g,:]> / sqrt(D)     (same-position only)
    probs = softmax_g(scores)
    o[t,h,:] = sum_g probs[t,h,g] * v[t,g,:]
    y = o @ Wo + bo

Strategy: data-parallel over the 16384 flattened tokens -> 2048 tokens/core
on 8 cores, all weights replicated, no collectives.  The matmuls run as
fp8-e4m3 DoubleRow (2 contraction blocks per instruction, 0.5 cyc/row ->
4x bf16 MAC rate), with compensated splits to stay inside the 2e-2 gate:
  - x is shipped as an fp8 (hi, lo) pair: xl = fp8(x - fp8(x)).
  - Q/K projections: (xh + xl) @ fp8(512 W)  - 2 chain segments each; the
    remaining weight-quantization noise only reaches the output through the
    4-way softmax, measured ~1.1e-2 end to end.
  - V projection: xh@Wvh + xl@Wvh + xh@Wvl  (weights split hi/lo) - ~exact.
  - attention middle on DVE/ACT exactly as before (bf16 staging, ~0.1%).
  - O^T (bf16, via DMA-xbar transpose) is split on-chip into fp8 hi/lo
    (ACT cast + DVE subtract) and o @ Wo runs the same 3-chain compensated
    form.  Scale bookkeeping: weights x512, o x16, exp scale /512^2,
    y copy /8192.
Per tile the PE does 51200 cycles (vs 81920 bf16) -> ~341us across 16
tiles; weight DMA (15 MiB fp8) overlaps the 2-tile prologue like before.
"""

import os
import sys

import numpy as np

sys.path.insert(0, "/opt/trn_rl_repo")

import ml_dtypes  # noqa: E402
from contextlib import ExitStack  # noqa: E402

import concourse.bass as bass  # noqa: E402
import concourse.bacc as bacc  # noqa: E402
import concourse.mybir as mybir  # noqa: E402
import concourse.tile as tile  # noqa: E402
from concourse.bass import ds  # noqa: E402
from concourse.bass_utils import run_bass_kernel_spmd  # noqa: E402
from concourse.masks import make_identity  # noqa: E402

B, S, HID = 4, 4096, 2048
H, G = 16, 4
D = HID // H          # 128
KV = HID * G // H     # 512
NCORES = 8
NTOK = B * S          # 16384
TPC = NTOK // NCORES  # 2048 tokens per core
P = 128
NTT = TPC // P        # 16 token tiles per core
NI = HID // P         # 16 input-feature blocks
NSP = NI // 2         # 8 DoubleRow step-pairs over the contraction
SCALE = 1.0 / float(np.sqrt(D))
WS = 512.0            # weight fp8 scale
OS = 16.0             # o fp8 scale

BF16 = mybir.dt.bfloat16
F32 = mybir.dt.float32
F8 = mybir.dt.float8e4
DR = mybir.MatmulPerfMode.DoubleRow

_cache = {}


def _build_fp8() -> bass.Bass:
    """No-bias fast path: fp8 DoubleRow matmuls with compensated splits."""
    nc = bacc.Bacc("TRN2")
    # xh/xl: host-pretransposed per token tile: row (t*128+p), col (i*128+tok)
    # holds x[t*128+tok, i*128+p]  -> per tile a plain [128, 2048] slice whose
    # block i is the lhsT [feat-in-block, token] for the QKV matmuls.
    xh = nc.dram_tensor("xh", [TPC, HID], F8, kind="ExternalInput")
    xl = nc.dram_tensor("xl", [TPC, HID], F8, kind="ExternalInput")
    wq = nc.dram_tensor("wq", [HID, HID], F8, kind="ExternalInput")
    wk = nc.dram_tensor("wk", [HID, KV], F8, kind="ExternalInput")
    wvh = nc.dram_tensor("wvh", [HID, KV], F8, kind="ExternalInput")
    wvl = nc.dram_tensor("wvl", [HID, KV], F8, kind="ExternalInput")
    woh = nc.dram_tensor("woh", [HID, HID], F8, kind="ExternalInput")
    wol = nc.dram_tensor("wol", [HID, HID], F8, kind="ExternalInput")
    y = nc.dram_tensor("y", [TPC, HID], BF16, kind="ExternalOutput")

    with tile.TileContext(nc) as tc, ExitStack() as ctx:
        w_pool = ctx.enter_context(tc.tile_pool(name="w", bufs=1))
        xh_pool = ctx.enter_context(tc.tile_pool(name="xh", bufs=4))
        xl_pool = ctx.enter_context(tc.tile_pool(name="xl", bufs=4))
        ysb_pool = ctx.enter_context(tc.tile_pool(name="ysb", bufs=2))
        qkv_ps_pool = ctx.enter_context(
            tc.tile_pool(name="qkvps", bufs=4, space="PSUM"))
        y_ps_pool = ctx.enter_context(tc.tile_pool(name="yps", bufs=4, space="PSUM"))
        qsb_pool = ctx.enter_context(tc.tile_pool(name="qsb", bufs=3))
        sm_pool = ctx.enter_context(tc.tile_pool(name="sm", bufs=2))
        wt_pool = ctx.enter_context(tc.tile_pool(name="wt", bufs=1))
        obf_pool = ctx.enter_context(tc.tile_pool(name="obf", bufs=2))
        ot_pool = ctx.enter_context(tc.tile_pool(name="ot", bufs=1))
        oth_pool = ctx.enter_context(tc.tile_pool(name="oth", bufs=4))
        otl_pool = ctx.enter_context(tc.tile_pool(name="otl", bufs=4))

        xh_sb = [None] * NTT
        xl_sb = [None] * NTT
        sm_sb = [None] * NTT
        obf_sb = [None] * NTT
        ot_sb = [None] * NTT
        oth_sb = [None] * NTT
        otl_sb = [None] * NTT

        def load_xh(t):
            xh_sb[t] = xh_pool.tile([P, HID], F8, name="xh", tag="xh")
            nc.sync.dma_start(xh_sb[t][:], xh[t * P:(t + 1) * P, :])

        def load_xl(t):
            xl_sb[t] = xl_pool.tile([P, HID], F8, name="xl", tag="xl")
            nc.sync.dma_start(xl_sb[t][:], xl[t * P:(t + 1) * P, :])

        # DMA emission order is the serial-DMA schedule: weights arrive in
        # the prologue's consumption order (chunked by contraction block) so
        # the DMA-bound startup overlaps the first two tiles' compute.
        xh_sb[0] = xh_pool.tile([P, HID], F8, name="xh", tag="xh")
        nc.sync.dma_start(xh_sb[0][:, 0:2 * P], xh[0:P, 0:2 * P])
        nc.sync.dma_start(xh_sb[0][:, 2 * P:], xh[0:P, 2 * P:])
        load_xh(1)

        wqa = w_pool.tile([P, NI * HID], F8, tag="wqa", name="wqa")
        wka = w_pool.tile([P, NI * KV], F8, tag="wka", name="wka")
        wvha = w_pool.tile([P, NI * KV], F8, tag="wvha", name="wvha")
        wvla = w_pool.tile([P, NI * KV], F8, tag="wvla", name="wvla")
        woha = w_pool.tile([P, NI * HID], F8, tag="woha", name="woha")
        wola = w_pool.tile([P, NI * HID], F8, tag="wola", name="wola")

        def wchunk(dst_all, src, i0, i1, c0, c1, w_):
            # weight blocks i0..i1-1, cols c0:c1, one DMA
            nc.sync.dma_start(
                dst_all[:, i0 * w_:i1 * w_]
                .rearrange("p (i c) -> p i c", c=w_)[:, :, c0:c1],
                src[i0 * P:i1 * P, c0:c1]
                .rearrange("(i p) c -> p i c", p=P),
            )

        # Prologue covers 4 tiles (phases A/B/C below): with fp8 DoubleRow
        # the PE retires weight bytes 4x faster than bf16, so only >=4
        # concurrent reader tiles keep the serial weight DMA stream off the
        # critical path.  2-block chunks match the chains' pair consumption;
        # xl0-3 land mid-stream (xl needed from chain step 8).
        PAIRS = [(2 * j, 2 * j + 2) for j in range(NSP)]
        load_xh(2)
        # phase A weights: wq cols 0:1024
        wchunk(wqa, wq, 0, 2, 0, 1024, HID)
        load_xl(0)
        wchunk(wqa, wq, 2, 4, 0, 1024, HID)
        load_xl(1)
        wchunk(wqa, wq, 4, 6, 0, 1024, HID)
        load_xl(2)
        for i0, i1 in PAIRS[3:]:
            wchunk(wqa, wq, i0, i1, 0, 1024, HID)
        # phase B weights: wq cols 1024:2048
        for i0, i1 in PAIRS:
            wchunk(wqa, wq, i0, i1, 1024, 2048, HID)
        # phase C weights: all of wk (its phase runs first so tile 0's
        # scores start early), then wvh, then wvl (consumed only from chain
        # step 16 of the V chains)
        for i0, i1 in PAIRS:
            wchunk(wka, wk, i0, i1, 0, KV, KV)
        for i0, i1 in PAIRS:
            wchunk(wvha, wvh, i0, i1, 0, KV, KV)
        for i0, i1 in PAIRS:
            wchunk(wvla, wvl, i0, i1, 0, KV, KV)
        # first half of wo-hi rides the phase-C DMA slack (phase C PE time
        # well exceeds its own weight bytes)
        for j in range(4):
            wchunk(woha, woh, 4 * j, 4 * (j + 1), 0, 1024, HID)

        wq_r = wqa[:].rearrange("p (i c) -> p i c", c=HID)
        wk_r = wka[:].rearrange("p (i c) -> p i c", c=KV)
        wvh_r = wvha[:].rearrange("p (i c) -> p i c", c=KV)
        wvl_r = wvla[:].rearrange("p (i c) -> p i c", c=KV)
        woh_r = woha[:].rearrange("p (i c) -> p i c", c=HID)
        wol_r = wola[:].rearrange("p (i c) -> p i c", c=HID)

        def xpair(t, s, lo):
            src = xl_sb[t] if lo else xh_sb[t]
            return src[:, 2 * s * P:(2 * s + 2) * P].rearrange(
                "p (i t2) -> p i t2", t2=P)

        def qkv_steps(t, which, c=0):
            """DoubleRow (lhsT, rhs) step list for one 512-col psum chain."""
            if which == "q":
                segs = [(False, wq_r), (True, wq_r)]
                cs = slice(c * 512, (c + 1) * 512)
            elif which == "k":
                segs = [(False, wk_r), (True, wk_r)]
                cs = slice(0, KV)
            else:  # v
                segs = [(False, wvh_r), (True, wvh_r), (False, wvl_r)]
                cs = slice(0, KV)
            steps = []
            for lo, w_r in segs:
                for s in range(NSP):
                    steps.append((xpair(t, s, lo), w_r[:, 2 * s:2 * s + 2, cs]))
            return steps

        def opair(t, s, lo):
            src = otl_sb[t] if lo else oth_sb[t]
            return src[:, 2 * s * P:(2 * s + 2) * P].rearrange(
                "p (o t2) -> p o t2", t2=P)

        def y_steps(tw, c0, c1):
            # o-half-major over the woh segments (so the chain starts as
            # soon as the first o-split half lands); the wol segment stays
            # last - it is the final arrival in the weight DMA stream
            cs = slice(c0, c1)
            steps = []
            for half in range(2):
                for lo in (False, True):
                    w_r = woh_r
                    for s in range(half * NSP // 2, (half + 1) * NSP // 2):
                        steps.append(
                            (opair(tw, s, lo), w_r[:, 2 * s:2 * s + 2, cs]))
            for s in range(NSP):
                steps.append((opair(tw, s, False), wol_r[:, 2 * s:2 * s + 2, cs]))
            return steps

        def emit_chain_pair(pairs):
            """pairs: list of (psum, steps[, on_done]); interleave
            step-wise, firing on_done(ps) right after a chain's stop so its
            psum copy is emitted as early as possible."""
            n = max(len(p[1]) for p in pairs)
            for s in range(n):
                for p in pairs:
                    ps, st = p[0], p[1]
                    if s < len(st):
                        lhs, rhs = st[s]
                        nc.tensor.matmul(
                            ps[:], lhs, rhs,
                            start=(s == 0), stop=(s == len(st) - 1),
                            perf_mode=DR,
                        )
                        if s == len(st) - 1 and len(p) > 2:
                            p[2](ps)

        HH = H // 2
        HW = HH * D  # 1024, half of the o columns

        def attn_half(t, qsb, ksb, vsb, hf):
            """scores + softmax + weighted sum for 8 heads (DVE + ACT +
            Pool).  Halving lets the o-split and the first y chains start
            one half-attn earlier."""
            if hf == 0:
                sm_sb[t] = (
                    sm_pool.tile([P, H * G], F32, tag="sc", name="sc"),
                    sm_pool.tile([P, H * G], F32, tag="ex", name="ex"),
                    sm_pool.tile([P, H], F32, tag="dn", name="dn"),
                    sm_pool.tile([P, H], F32, tag="rc", name="rc"),
                    sm_pool.tile([P, H * G], F32, tag="pf", name="pf"),
                    sm_pool.tile([P, D], BF16, tag="junk", name="junk"),
                )
                obf_sb[t] = obf_pool.tile([P, HID], BF16, name="obf",
                                          tag="obf")
            sc, ex, dn, rc, pf, junk = sm_sb[t]
            obf = obf_sb[t]
            h0 = hf * HH
            cs = slice(h0 * G, (h0 + HH) * G)

            # raw scores sc[t,(h,g)] = <q_h, k_g>  (fused mult+reduce, DVE)
            for h in range(h0, h0 + HH):
                for g in range(G):
                    nc.vector.scalar_tensor_tensor(
                        junk[:],
                        qsb[:, h * D:(h + 1) * D],
                        1.0,
                        ksb[:, g * D:(g + 1) * D],
                        op0=mybir.AluOpType.mult,
                        op1=mybir.AluOpType.mult,
                        accum_out=sc[:, ds(h * G + g, 1)],
                    )

            # softmax over g; q,k carry x512 each -> exp scale /512^2
            nc.scalar.activation(
                ex[:, cs], sc[:, cs], mybir.ActivationFunctionType.Exp,
                scale=SCALE / (WS * WS))
            nc.vector.reduce_sum(
                dn[:, h0:h0 + HH],
                ex[:, cs].rearrange("p (h g) -> p h g", g=G),
                axis=mybir.AxisListType.X,
            )
            nc.vector.reciprocal(rc[:, h0:h0 + HH], dn[:, h0:h0 + HH])
            # pf = OS * ex / dn  (o shipped as 16*o for the fp8 split)
            nc.vector.scalar_tensor_tensor(
                pf[:, cs].rearrange("p (h g) -> p h g", g=G),
                ex[:, cs].rearrange("p (h g) -> p h g", g=G),
                OS,
                rc[:, h0:h0 + HH].unsqueeze(2).broadcast_to((P, HH, G)),
                op0=mybir.AluOpType.mult, op1=mybir.AluOpType.mult,
            )

            # o[t,(h,d)] = sum_g p[t,(h,g)] * v[t,(g,d)]  - batched per g on
            # the otherwise-idle GpSimd/Pool engine (4 mult + 3 accum ops of
            # [p, HH*D], broadcast views; DVE keeps only the score dots)
            tmp = wt_pool.tile([P, HW], BF16, tag="ta", name="ta")
            tm3 = tmp[:].rearrange("p (h d) -> p h d", d=D)
            ob3 = obf[:, hf * HW:(hf + 1) * HW] \
                .rearrange("p (h d) -> p h d", d=D)
            pfh = pf[:].rearrange("p (h g) -> p h g", g=G)[:, h0:h0 + HH, :]
            vv = lambda g: vsb[:, g * D:(g + 1) * D].unsqueeze(1) \
                .broadcast_to((P, HH, D))
            pp = lambda g: pfh[:, :, g:g + 1].broadcast_to((P, HH, D))
            nc.gpsimd.tensor_tensor(ob3, vv(0), pp(0),
                                    op=mybir.AluOpType.mult)
            for g in range(1, G):
                nc.gpsimd.tensor_tensor(tm3, vv(g), pp(g),
                                        op=mybir.AluOpType.mult)
                nc.gpsimd.tensor_tensor(ob3, ob3, tm3,
                                        op=mybir.AluOpType.add)

        def attn_middle(t, qsb, ksb, vsb):
            attn_half(t, qsb, ksb, vsb, 0)
            attn_half(t, qsb, ksb, vsb, 1)

        def split_half(t, hf):
            # O^T via the DMA xbar (keeps the PE matmul-only):
            # ot[p, o*128+tok] = obf[tok, o*128+p]; then split to fp8 hi/lo.
            if hf == 0:
                ot_sb[t] = ot_pool.tile([P, HID], BF16, name="ot", tag="ot")
                oth_sb[t] = oth_pool.tile([P, HID], F8, name="oth",
                                          tag="oth")
                otl_sb[t] = otl_pool.tile([P, HID], F8, name="otl",
                                          tag="otl")
            ot, oth, otl = ot_sb[t], oth_sb[t], otl_sb[t]
            hs = slice(hf * HW, (hf + 1) * HW)
            nc.sync.dma_start_transpose(
                ot[:, hs].rearrange("p (o t2) -> p o t2", t2=P),
                obf_sb[t][:, hs])
            # both split ops on Pool: ACT stays free of transpose-DMA
            # dependencies, so the PE's psum copies never queue behind them
            nc.gpsimd.tensor_copy(oth[:, hs], ot[:, hs])
            nc.gpsimd.tensor_sub(otl[:, hs], ot[:, hs], oth[:, hs])

        def transpose_split_o(t):
            split_half(t, 0)
            split_half(t, 1)
            obf_sb[t] = None
            ot_sb[t] = None

        def copy_qkv(which, c, ps, qsb, ksb, vsb):
            if which == "q":
                nc.scalar.copy(qsb[:, c * 512:(c + 1) * 512], ps[:])
            elif which == "k":
                nc.scalar.copy(ksb[:], ps[:])
            else:
                nc.scalar.mul(vsb[:], ps[:], 1.0 / WS)

        def emit_y_tile(tw):
            if tw == NTT - 1:
                # last tile: sequential chains, final ones narrow, so the
                # post-matmul drain holds only one short copy+DMA
                for c0, c1 in ((0, 512), (512, 1024), (1024, 1536),
                               (1536, 1792), (1792, 2048)):
                    yps = y_ps_pool.tile([P, c1 - c0], F32,
                                         name="yps", tag="yps")
                    emit_chain_pair([(yps, y_steps(tw, c0, c1))])
                    ysb = ysb_pool.tile([P, c1 - c0], BF16,
                                        name="ysb", tag="ysb")
                    nc.scalar.mul(ysb[:], yps[:], 1.0 / (WS * OS))
                    nc.sync.dma_start(
                        y[tw * P:(tw + 1) * P, c0:c1], ysb[:])
                oth_sb[tw] = None
                otl_sb[tw] = None
                return
            for sp in range(2):
                ypa = y_ps_pool.tile([P, 512], F32, name="yps", tag="yps")
                ypb = y_ps_pool.tile([P, 512], F32, name="yps", tag="yps")
                emit_chain_pair([
                    (ypa, y_steps(tw, 2 * sp * 512, (2 * sp + 1) * 512)),
                    (ypb, y_steps(tw, (2 * sp + 1) * 512, (2 * sp + 2) * 512)),
                ])
                for yps, s_ in ((ypa, 2 * sp), (ypb, 2 * sp + 1)):
                    ysb = ysb_pool.tile([P, 512], BF16, name="ysb", tag="ysb")
                    nc.scalar.mul(ysb[:], yps[:], 1.0 / (WS * OS))
                    nc.sync.dma_start(
                        y[tw * P:(tw + 1) * P, s_ * 512:(s_ + 1) * 512],
                        ysb[:])
            oth_sb[tw] = None
            otl_sb[tw] = None

        # ---- prologue: tiles 0-3 in three phases, each running 8 psum
        # chains (all banks) step-major so every weight chunk is consumed
        # by 4 reader tiles as it arrives ----
        NPT = 3
        pro_sb = {}
        for tt in range(NPT):
            pro_sb[tt] = (
                qsb_pool.tile([P, HID], BF16, tag="q", name="q"),
                qsb_pool.tile([P, KV], BF16, tag="k", name="k"),
                qsb_pool.tile([P, KV], BF16, tag="v", name="v"),
            )
        for chains in ([("q", 0), ("q", 1)], [("q", 2), ("q", 3)],
                       [("k", 0)], [("v", 0)]):
            pairs = []
            for ci, (which, c) in enumerate(chains):
                for tt in range(NPT):
                    pool = qkv_ps_pool if ci == 0 else y_ps_pool
                    tagname = "ps" if ci == 0 else "yps"
                    ps = pool.tile([P, 512], F32, name=tagname,
                                   tag=tagname)
                    pairs.append(
                        (ps, qkv_steps(tt, which, c),
                         (lambda w_, c_, t_: lambda psd: copy_qkv(
                             w_, c_, psd, *pro_sb[t_]))(which, c, tt)))
            emit_chain_pair(pairs)
        # attn + o-split interleaved per half-tile so each split sits
        # directly behind its weighted-sum in the Pool queue and the first
        # y chains start after only half an attn
        for tt in range(NPT):
            for hf in range(2):
                attn_half(tt, *pro_sb[tt], hf)
                split_half(tt, hf)
            obf_sb[tt] = None
            ot_sb[tt] = None
        load_xh(3)
        load_xl(3)
        load_xh(4)
        load_xl(4)

        # ---- steady state: QKV(t) | Wo(t-3 or t-4); the wo weight stream
        # is emitted at the first loop iteration, after every qkv weight and
        # the early x tiles, so the first y chains never wait on it ----
        for t in range(NPT, NTT + 4):
            if t <= NTT - 3:
                load_xh(t + 2)
                load_xl(t + 2)
            if t == NPT:
                # rest of wo, in Y(0)'s consumption order: its first chain
                # pair tails with wol cols 0:1024, then the second pair
                # opens on woh cols 1024:2048
                for j in range(4):
                    wchunk(wola, wol, 4 * j, 4 * (j + 1), 0, 1024, HID)
                for j in range(4):
                    wchunk(woha, woh, 4 * j, 4 * (j + 1), 1024, 2048, HID)
                for j in range(4):
                    wchunk(wola, wol, 4 * j, 4 * (j + 1), 1024, 2048, HID)

            if t < NTT:
                qsb = qsb_pool.tile([P, HID], BF16, tag="q")
                ksb = qsb_pool.tile([P, KV], BF16, tag="k")
                vsb = qsb_pool.tile([P, KV], BF16, tag="v")

                # 3 pair-interleaved chains; paired chains share the
                # stationary x pair per step (back-to-back reuse)
                for pi, (pa, pb) in enumerate(((("q", 0), ("k", 0)),
                                               (("q", 1), ("q", 2)),
                                               (("q", 3), ("v", 0)))):
                    pool = y_ps_pool if (t == NPT and pi == 2) \
                        else qkv_ps_pool
                    tg = "yps" if (t == NPT and pi == 2) else "ps"
                    psa = pool.tile([P, 512], F32, name=tg, tag=tg)
                    psb = pool.tile([P, 512], F32, name=tg, tag=tg)
                    emit_chain_pair([
                        (psa, qkv_steps(t, pa[0], pa[1])),
                        (psb, qkv_steps(t, pb[0], pb[1])),
                    ])
                    copy_qkv(pa[0], pa[1], psa, qsb, ksb, vsb)
                    copy_qkv(pb[0], pb[1], psb, qsb, ksb, vsb)

                attn_middle(t, qsb, ksb, vsb)

            # Wo matmuls + y out, lagged 5 tiles behind the QKV stream so
            # the first y chains never wait on the prologue's attn backlog
            # (4 tiles of DVE/Pool work drain at ~8us/iter) nor on the
            # wo weight DMA
            if t - 4 >= 0:
                emit_y_tile(t - 4)

            if NPT <= t < NTT:
                transpose_split_o(t)

    nc.compile()
    return nc


def _build_bias(has_bias: bool = True) -> bass.Bass:
    """Original (slower) path, kept for the biased case."""
    nc = bacc.Bacc("TRN2")
    x = nc.dram_tensor("x", [TPC, HID], BF16, kind="ExternalInput")
    wq = nc.dram_tensor("wq", [HID, HID], BF16, kind="ExternalInput")
    wk = nc.dram_tensor("wk", [HID, KV], BF16, kind="ExternalInput")
    wv = nc.dram_tensor("wv", [HID, KV], BF16, kind="ExternalInput")
    wo = nc.dram_tensor("wo", [HID, HID], BF16, kind="ExternalInput")
    if has_bias:
        bqkv = nc.dram_tensor("bqkv", [1, HID + 2 * KV], F32, kind="ExternalInput")
        bo = nc.dram_tensor("bo", [1, HID], F32, kind="ExternalInput")
    y = nc.dram_tensor("y", [TPC, HID], F32, kind="ExternalOutput")

    with tile.TileContext(nc) as tc, ExitStack() as ctx:
        const_pool = ctx.enter_context(tc.tile_pool(name="const", bufs=1))
        ident = const_pool.tile([P, P], BF16)
        make_identity(nc, ident[:])

        if has_bias:
            bias_qkv = const_pool.tile([P, HID + 2 * KV], F32)
            nc.sync.dma_start(bias_qkv[:], bqkv[0:1, :].broadcast_to((P, HID + 2 * KV)))
            bias_o = const_pool.tile([P, HID], F32)
            nc.sync.dma_start(bias_o[:], bo[0:1, :].broadcast_to((P, HID)))

        # O^T staging for the whole core: [o_block(16) x tokens(2048)] bf16
        ofm_pool = ctx.enter_context(tc.tile_pool(name="ofm", bufs=1))
        ofm = ofm_pool.tile([P, NI * TPC], BF16)

        kv_pool = ctx.enter_context(tc.tile_pool(name="wkv", bufs=1))
        wk_sb = []
        wv_sb = []
        for i in range(NI):
            wk_t = kv_pool.tile([P, KV], BF16, tag=f"wk{i}")
            nc.sync.dma_start(wk_t[:], wk[i * P:(i + 1) * P, :])
            wk_sb.append(wk_t)
            wv_t = kv_pool.tile([P, KV], BF16, tag=f"wv{i}")
            nc.sync.dma_start(wv_t[:], wv[i * P:(i + 1) * P, :])
            wv_sb.append(wv_t)

        pt_pool = ctx.enter_context(tc.tile_pool(name="pt", bufs=2, space="PSUM"))
        mm_pool = ctx.enter_context(tc.tile_pool(name="mm", bufs=3, space="PSUM"))

        # ---------------- Phase A: QKV projections + attention ----------------
        with tc.tile_pool(name="wqp", bufs=1) as wq_pool, \
             tc.tile_pool(name="xt", bufs=3) as xt_pool, \
             tc.tile_pool(name="xfm", bufs=1) as xfm_pool, \
             tc.tile_pool(name="qkv", bufs=1) as qkv_pool, \
             tc.tile_pool(name="attn", bufs=2) as attn_pool, \
             tc.tile_pool(name="oacc", bufs=1) as oacc_pool, \
             tc.tile_pool(name="obf", bufs=1) as obf_pool:
            wq_sb = []
            for i in range(NI):
                wq_t = wq_pool.tile([P, HID], BF16, tag=f"wq{i}")
                nc.sync.dma_start(wq_t[:], wq[i * P:(i + 1) * P, :])
                wq_sb.append(wq_t)

            for t in range(NTT):
                xt = xt_pool.tile([P, HID], BF16)
                nc.sync.dma_start(xt[:], x[t * P:(t + 1) * P, :])

                # transpose X tile to feature-major [i, t] (16 blocks of 128x128)
                xfm = xfm_pool.tile([P, HID], BF16)
                for j in range(4):
                    pt = pt_pool.tile([P, 512], BF16)
                    for k in range(4):
                        blk = 4 * j + k
                        nc.tensor.transpose(
                            pt[:, k * P:(k + 1) * P],
                            xt[:, blk * P:(blk + 1) * P],
                            ident[:],
                        )
                    nc.vector.tensor_copy(xfm[:, j * 512:(j + 1) * 512], pt[:])

                # QKV projections, token-major out: [t(128part), 3072]
                qkv = qkv_pool.tile([P, HID + 2 * KV], F32)
                for s in range(6):
                    ps = mm_pool.tile([P, 512], F32)
                    for i in range(NI):
                        if s < 4:
                            rhs = wq_sb[i][:, s * 512:(s + 1) * 512]
                        elif s == 4:
                            rhs = wk_sb[i][:]
                        else:
                            rhs = wv_sb[i][:]
                        nc.tensor.matmul(
                            ps[:], xfm[:, i * P:(i + 1) * P], rhs,
                            start=(i == 0), stop=(i == NI - 1),
                        )
                    if has_bias:
                        nc.vector.tensor_add(
                            qkv[:, s * 512:(s + 1) * 512], ps[:],
                            bias_qkv[:, s * 512:(s + 1) * 512],
                        )
                    else:
                        nc.vector.tensor_copy(qkv[:, s * 512:(s + 1) * 512], ps[:])

                # scores[t, h, g] = <q_h, k_g> * SCALE   (fused mult+reduce)
                sc = attn_pool.tile([P, H * G], F32, tag="sc")
                junk = attn_pool.tile([P, D], F32, tag="junk")
                for h in range(H):
                    for g in range(G):
                        nc.vector.scalar_tensor_tensor(
                            junk[:],
                            qkv[:, h * D:(h + 1) * D],
                            SCALE,
                            qkv[:, HID + g * D:HID + (g + 1) * D],
                            op0=mybir.AluOpType.mult,
                            op1=mybir.AluOpType.mult,
                            accum_out=sc[:, ds(h * G + g, 1)],
                        )

                # softmax over g (4); denominator folded into final scale
                ex = attn_pool.tile([P, H * G], F32, tag="ex")
                nc.scalar.activation(ex[:], sc[:], mybir.ActivationFunctionType.Exp)
                dn = attn_pool.tile([P, H], F32, tag="dn")
                nc.vector.reduce_sum(
                    dn[:], ex[:].rearrange("p (h g) -> p h g", g=G),
                    axis=mybir.AxisListType.X,
                )
                rc = attn_pool.tile([P, H], F32, tag="rc")
                nc.vector.reciprocal(rc[:], dn[:])

                # o[t, h*D+d] = (sum_g ex[t,h,g] * v[t, g*D+d]) * rc[t,h]
                acc = oacc_pool.tile([P, HID], F32, tag="acc")
                tmp = oacc_pool.tile([P, HID], F32, tag="tmp")
                obf = obf_pool.tile([P, HID], BF16)
                ab = [acc, tmp]
                for h in range(H):
                    hs = ds(h * D, D)
                    nc.vector.tensor_scalar_mul(
                        ab[0][:, hs],
                        qkv[:, HID + KV:HID + KV + D],
                        ex[:, ds(h * G, 1)],
                    )
                    for g in range(1, G):
                        nc.vector.scalar_tensor_tensor(
                            ab[g % 2][:, hs],
                            qkv[:, HID + KV + g * D:HID + KV + (g + 1) * D],
                            ex[:, ds(h * G + g, 1)],
                            ab[(g - 1) % 2][:, hs],
                            op0=mybir.AluOpType.mult,
                            op1=mybir.AluOpType.add,
                        )
                    nc.vector.tensor_scalar_mul(
                        obf[:, hs], ab[(G - 1) % 2][:, hs], rc[:, ds(h, 1)])

                # transpose O tile into ofm [o_block, token]
                for j in range(4):
                    pt = pt_pool.tile([P, 512], BF16)
                    for k in range(4):
                        blk = 4 * j + k
                        nc.tensor.transpose(
                            pt[:, k * P:(k + 1) * P],
                            obf[:, blk * P:(blk + 1) * P],
                            ident[:],
                        )
                    nc.vector.tensor_copy(
                        ofm[:].rearrange("p (o t) -> p o t", t=TPC)
                              [:, 4 * j:4 * j + 4, t * P:(t + 1) * P],
                        pt[:].rearrange("p (o t) -> p o t", t=P),
                    )

        # ---------------- Phase B: output projection ----------------
        with tc.tile_pool(name="wop", bufs=1) as wo_pool, \
             tc.tile_pool(name="yt", bufs=3) as yt_pool:
            wo_sb = []
            for i in range(NI):
                wo_t = wo_pool.tile([P, HID], BF16, tag=f"wo{i}")
                nc.sync.dma_start(wo_t[:], wo[i * P:(i + 1) * P, :])
                wo_sb.append(wo_t)

            for t in range(NTT):
                for s in range(4):
                    ps = mm_pool.tile([P, 512], F32)
                    for o in range(NI):
                        nc.tensor.matmul(
                            ps[:],
                            ofm[:, ds(o * TPC + t * P, P)],
                            wo_sb[o][:, s * 512:(s + 1) * 512],
                            start=(o == 0), stop=(o == NI - 1),
                        )
                    yt = yt_pool.tile([P, 512], F32)
                    if has_bias:
                        nc.vector.tensor_add(
                            yt[:], ps[:], bias_o[:, s * 512:(s + 1) * 512])
                    else:
                        nc.vector.tensor_copy(yt[:], ps[:])
                    nc.sync.dma_start(
                        y[t * P:(t + 1) * P, s * 512:(s + 1) * 512], yt[:])

    nc.compile()
    return nc


def _build(has_bias: bool) -> bass.Bass:
    return _build_bias(True) if has_bias else _build_fp8()


def kernel(hidden_states, Wq, bq, Wk, bk, Wv, bv, Wo, bo, _profile=None):
    has_bias = bool(np.any(bq) or np.any(bk) or np.any(bv) or np.any(bo))
    key = has_bias
    if key not in _cache:
        _cache[key] = _build(has_bias)
    nc = _cache[key]

    x_flat = np.ascontiguousarray(
        np.asarray(hidden_states, dtype=np.float32).reshape(NTOK, HID))

    in_maps = []
    if has_bias:
        bf = ml_dtypes.bfloat16
        xb = x_flat.astype(bf)
        wq_b = np.asarray(Wq, dtype=np.float32).astype(bf)
        wk_b = np.asarray(Wk, dtype=np.float32).astype(bf)
        wv_b = np.asarray(Wv, dtype=np.float32).astype(bf)
        wo_b = np.asarray(Wo, dtype=np.float32).astype(bf)
        for c in range(NCORES):
            m = {
                "x": np.ascontiguousarray(xb[c * TPC:(c + 1) * TPC]),
                "wq": wq_b, "wk": wk_b, "wv": wv_b, "wo": wo_b,
                "bqkv": np.concatenate([
                    np.asarray(bq, np.float32), np.asarray(bk, np.float32),
                    np.asarray(bv, np.float32)]).reshape(1, HID + 2 * KV),
                "bo": np.asarray(bo, np.float32).reshape(1, HID),
            }
            in_maps.append(m)
    else:
        e4 = ml_dtypes.float8_e4m3
        xh8 = x_flat.astype(e4)
        xl8 = (x_flat - xh8.astype(np.float32)).astype(e4)

        def wsplit(W):
            Wf = np.asarray(W, dtype=np.float32) * WS
            hi = Wf.astype(e4)
            lo = (Wf - hi.astype(np.float32)).astype(e4)
            return np.ascontiguousarray(hi), np.ascontiguousarray(lo)

        wq8 = np.ascontiguousarray(
            (np.asarray(Wq, np.float32) * WS).astype(e4))
        wk8 = np.ascontiguousarray(
            (np.asarray(Wk, np.float32) * WS).astype(e4))
        wvh8, wvl8 = wsplit(Wv)
        woh8, wol8 = wsplit(Wo)

        def pret(a):
            # host pre-transpose: row (t*128+p), col (i*128+tok) <- x[(t,tok),(i,p)]
            return np.ascontiguousarray(
                a.reshape(NTT, P, NI, P).transpose(0, 3, 2, 1).reshape(TPC, HID))

        for c in range(NCORES):
            m = {
                "xh": pret(xh8[c * TPC:(c + 1) * TPC]),
                "xl": pret(xl8[c * TPC:(c + 1) * TPC]),
                "wq": wq8, "wk": wk8,
                "wvh": wvh8, "wvl": wvl8,
                "woh": woh8, "wol": wol8,
            }
            in_maps.append(m)

    kwargs = dict(_profile) if _profile else {}
    kwargs.pop("result", None)
    res = run_bass_kernel_spmd(nc, in_maps, list(range(NCORES)), **kwargs)
    out = np.concatenate([r["y"] for r in res.results], axis=0)
    if _profile is not None:
        _profile["result"] = res
    return out.reshape(B, S, HID).astype(np.float32)


# revision 25
# speedup vs baseline: 1.0287x; 1.0163x over previous
"""Trainium2 Bass kernel for per-position grouped-query attention.

Reference computation (B=4, S=4096, HID=2048, H=16, G=4, D=128, KV=512):
    q = x @ Wq + bq ; k = x @ Wk + bk ; v = x @ Wv + bv
    scores[t,h,g] = <q[t,h,:], k[t,g,:]> / sqrt(D)     (same-position only)
    probs = softmax_g(scores)
    o[t,h,:] = sum_g probs[t,h,g] * v[t,g,:]
    y = o @ Wo + bo

Strategy: data-parallel over the 16384 flattened tokens -> 2048 tokens/core
on 8 cores, all weights replicated, no collectives.  The matmuls run as
fp8-e4m3 DoubleRow (2 contraction blocks per instruction, 0.5 cyc/row ->
4x bf16 MAC rate), with compensated splits to stay inside the 2e-2 gate:
  - x is shipped as an fp8 (hi, lo) pair: xl = fp8(x - fp8(x)).
  - Q/K projections: (xh + xl) @ fp8(512 W)  - 2 chain segments each; the
    remaining weight-quantization noise only reaches the output through the
    4-way softmax, measured ~1.1e-2 end to end.
  - V projection: xh@Wvh + xl@Wvh + xh@Wvl  (weights split hi/lo) - ~exact.
  - attention middle on DVE/ACT exactly as before (bf16 staging, ~0.1%).
  - O^T (bf16, via DMA-xbar transpose) is split on-chip into fp8 hi/lo
    (ACT cast + DVE subtract) and o @ Wo runs the same 3-chain compensated
    form.  Scale bookkeeping: weights x512, o x16, exp scale /512^2,
    y copy /8192.
Per tile the PE does 51200 cycles (vs 81920 bf16) -> ~341us across 16
tiles; weight DMA (15 MiB fp8) overlaps the 2-tile prologue like before.
"""

import os
import sys

import numpy as np

sys.path.insert(0, "/opt/trn_rl_repo")

import ml_dtypes  # noqa: E402
from contextlib import ExitStack  # noqa: E402

import concourse.bass as bass  # noqa: E402
import concourse.bacc as bacc  # noqa: E402
import concourse.mybir as mybir  # noqa: E402
import concourse.tile as tile  # noqa: E402
from concourse.bass import ds  # noqa: E402
from concourse.bass_utils import run_bass_kernel_spmd  # noqa: E402
from concourse.masks import make_identity  # noqa: E402

B, S, HID = 4, 4096, 2048
H, G = 16, 4
D = HID // H          # 128
KV = HID * G // H     # 512
NCORES = 8
NTOK = B * S          # 16384
TPC = NTOK // NCORES  # 2048 tokens per core
P = 128
NTT = TPC // P        # 16 token tiles per core
NI = HID // P         # 16 input-feature blocks
NSP = NI // 2         # 8 DoubleRow step-pairs over the contraction
SCALE = 1.0 / float(np.sqrt(D))
WS = 512.0            # weight fp8 scale
OS = 16.0             # o fp8 scale

BF16 = mybir.dt.bfloat16
F32 = mybir.dt.float32
F8 = mybir.dt.float8e4
DR = mybir.MatmulPerfMode.DoubleRow

_cache = {}


def _build_fp8() -> bass.Bass:
    """No-bias fast path: fp8 DoubleRow matmuls with compensated splits."""
    nc = bacc.Bacc("TRN2")
    # xh/xl: host-pretransposed per token tile: row (t*128+p), col (i*128+tok)
    # holds x[t*128+tok, i*128+p]  -> per tile a plain [128, 2048] slice whose
    # block i is the lhsT [feat-in-block, token] for the QKV matmuls.
    xh = nc.dram_tensor("xh", [TPC, HID], F8, kind="ExternalInput")
    xl = nc.dram_tensor("xl", [TPC, HID], F8, kind="ExternalInput")
    wq = nc.dram_tensor("wq", [HID, HID], F8, kind="ExternalInput")
    wk = nc.dram_tensor("wk", [HID, KV], F8, kind="ExternalInput")
    wvh = nc.dram_tensor("wvh", [HID, KV], F8, kind="ExternalInput")
    wvl = nc.dram_tensor("wvl", [HID, KV], F8, kind="ExternalInput")
    woh = nc.dram_tensor("woh", [HID, HID], F8, kind="ExternalInput")
    wol = nc.dram_tensor("wol", [HID, HID], F8, kind="ExternalInput")
    y = nc.dram_tensor("y", [TPC, HID], BF16, kind="ExternalOutput")

    with tile.TileContext(nc) as tc, ExitStack() as ctx:
        w_pool = ctx.enter_context(tc.tile_pool(name="w", bufs=1))
        xh_pool = ctx.enter_context(tc.tile_pool(name="xh", bufs=3))
        xl_pool = ctx.enter_context(tc.tile_pool(name="xl", bufs=3))
        ysb_pool = ctx.enter_context(tc.tile_pool(name="ysb", bufs=2))
        qkv_ps_pool = ctx.enter_context(
            tc.tile_pool(name="qkvps", bufs=4, space="PSUM"))
        y_ps_pool = ctx.enter_context(tc.tile_pool(name="yps", bufs=4, space="PSUM"))
        qsb_pool = ctx.enter_context(tc.tile_pool(name="qsb", bufs=4))
        sm_pool = ctx.enter_context(tc.tile_pool(name="sm", bufs=2))
        wt_pool = ctx.enter_context(tc.tile_pool(name="wt", bufs=1))
        obf_pool = ctx.enter_context(tc.tile_pool(name="obf", bufs=2))
        ot_pool = ctx.enter_context(tc.tile_pool(name="ot", bufs=1))
        oth_pool = ctx.enter_context(tc.tile_pool(name="oth", bufs=4))
        otl_pool = ctx.enter_context(tc.tile_pool(name="otl", bufs=4))

        xh_sb = [None] * NTT
        xl_sb = [None] * NTT
        sm_sb = [None] * NTT
        obf_sb = [None] * NTT
        ot_sb = [None] * NTT
        oth_sb = [None] * NTT
        otl_sb = [None] * NTT

        def load_xh(t):
            xh_sb[t] = xh_pool.tile([P, HID], F8, name="xh", tag="xh")
            nc.sync.dma_start(xh_sb[t][:], xh[t * P:(t + 1) * P, :])

        def load_xl(t):
            xl_sb[t] = xl_pool.tile([P, HID], F8, name="xl", tag="xl")
            nc.sync.dma_start(xl_sb[t][:], xl[t * P:(t + 1) * P, :])

        # DMA emission order is the serial-DMA schedule: weights arrive in
        # the prologue's consumption order (chunked by contraction block) so
        # the DMA-bound startup overlaps the first two tiles' compute.
        xh_sb[0] = xh_pool.tile([P, HID], F8, name="xh", tag="xh")
        nc.sync.dma_start(xh_sb[0][:, 0:2 * P], xh[0:P, 0:2 * P])
        nc.sync.dma_start(xh_sb[0][:, 2 * P:], xh[0:P, 2 * P:])
        load_xh(1)

        wqa = w_pool.tile([P, NI * HID], F8, tag="wqa", name="wqa")
        wka = w_pool.tile([P, NI * KV], F8, tag="wka", name="wka")
        wvha = w_pool.tile([P, NI * KV], F8, tag="wvha", name="wvha")
        wvla = w_pool.tile([P, NI * KV], F8, tag="wvla", name="wvla")
        woha = w_pool.tile([P, NI * HID], F8, tag="woha", name="woha")
        wola = w_pool.tile([P, NI * HID], F8, tag="wola", name="wola")

        def wchunk(dst_all, src, i0, i1, c0, c1, w_):
            # weight blocks i0..i1-1, cols c0:c1, one DMA
            nc.sync.dma_start(
                dst_all[:, i0 * w_:i1 * w_]
                .rearrange("p (i c) -> p i c", c=w_)[:, :, c0:c1],
                src[i0 * P:i1 * P, c0:c1]
                .rearrange("(i p) c -> p i c", p=P),
            )

        # Prologue covers 4 tiles (phases A/B/C below): with fp8 DoubleRow
        # the PE retires weight bytes 4x faster than bf16, so only >=4
        # concurrent reader tiles keep the serial weight DMA stream off the
        # critical path.  2-block chunks match the chains' pair consumption;
        # xl0-3 land mid-stream (xl needed from chain step 8).
        PAIRS = [(2 * j, 2 * j + 2) for j in range(NSP)]
        load_xh(2)
        # phase A weights: wq cols 0:1024
        wchunk(wqa, wq, 0, 2, 0, 1024, HID)
        load_xl(0)
        wchunk(wqa, wq, 2, 4, 0, 1024, HID)
        load_xl(1)
        wchunk(wqa, wq, 4, 6, 0, 1024, HID)
        load_xl(2)
        for i0, i1 in PAIRS[3:]:
            wchunk(wqa, wq, i0, i1, 0, 1024, HID)
        # phase B weights: wq cols 1024:2048
        for i0, i1 in PAIRS:
            wchunk(wqa, wq, i0, i1, 1024, 2048, HID)
        # phase C weights: all of wk (its phase runs first so tile 0's
        # scores start early), then wvh, then wvl (consumed only from chain
        # step 16 of the V chains)
        for i0, i1 in PAIRS:
            wchunk(wka, wk, i0, i1, 0, KV, KV)
        for i0, i1 in PAIRS:
            wchunk(wvha, wvh, i0, i1, 0, KV, KV)
        for i0, i1 in PAIRS:
            wchunk(wvla, wvl, i0, i1, 0, KV, KV)
        # first half of wo-hi rides the phase-C DMA slack (phase C PE time
        # well exceeds its own weight bytes)
        for j in range(4):
            wchunk(woha, woh, 4 * j, 4 * (j + 1), 0, 1024, HID)

        wq_r = wqa[:].rearrange("p (i c) -> p i c", c=HID)
        wk_r = wka[:].rearrange("p (i c) -> p i c", c=KV)
        wvh_r = wvha[:].rearrange("p (i c) -> p i c", c=KV)
        wvl_r = wvla[:].rearrange("p (i c) -> p i c", c=KV)
        woh_r = woha[:].rearrange("p (i c) -> p i c", c=HID)
        wol_r = wola[:].rearrange("p (i c) -> p i c", c=HID)

        def xpair(t, s, lo):
            src = xl_sb[t] if lo else xh_sb[t]
            return src[:, 2 * s * P:(2 * s + 2) * P].rearrange(
                "p (i t2) -> p i t2", t2=P)

        def qkv_steps(t, which, c=0):
            """DoubleRow (lhsT, rhs) step list for one 512-col psum chain."""
            if which == "q":
                segs = [(False, wq_r), (True, wq_r)]
                cs = slice(c * 512, (c + 1) * 512)
            elif which == "k":
                segs = [(False, wk_r), (True, wk_r)]
                cs = slice(0, KV)
            else:  # v
                segs = [(False, wvh_r), (True, wvh_r), (False, wvl_r)]
                cs = slice(0, KV)
            steps = []
            for lo, w_r in segs:
                for s in range(NSP):
                    steps.append((xpair(t, s, lo), w_r[:, 2 * s:2 * s + 2, cs]))
            return steps

        def opair(t, s, lo):
            src = otl_sb[t] if lo else oth_sb[t]
            return src[:, 2 * s * P:(2 * s + 2) * P].rearrange(
                "p (o t2) -> p o t2", t2=P)

        def y_steps(tw, c0, c1):
            # o-half-major over the woh segments (so the chain starts as
            # soon as the first o-split half lands); the wol segment stays
            # last - it is the final arrival in the weight DMA stream
            cs = slice(c0, c1)
            steps = []
            for half in range(2):
                for lo in (False, True):
                    w_r = woh_r
                    for s in range(half * NSP // 2, (half + 1) * NSP // 2):
                        steps.append(
                            (opair(tw, s, lo), w_r[:, 2 * s:2 * s + 2, cs]))
            for s in range(NSP):
                steps.append((opair(tw, s, False), wol_r[:, 2 * s:2 * s + 2, cs]))
            return steps

        def emit_chain_pair(pairs):
            """pairs: list of (psum, steps[, on_done]); interleave
            step-wise, firing on_done(ps) right after a chain's stop so its
            psum copy is emitted as early as possible."""
            n = max(len(p[1]) for p in pairs)
            for s in range(n):
                for p in pairs:
                    ps, st = p[0], p[1]
                    if s < len(st):
                        lhs, rhs = st[s]
                        nc.tensor.matmul(
                            ps[:], lhs, rhs,
                            start=(s == 0), stop=(s == len(st) - 1),
                            perf_mode=DR,
                        )
                        if s == len(st) - 1 and len(p) > 2:
                            p[2](ps)

        HH = H // 2
        HW = HH * D  # 1024, half of the o columns

        def attn_half(t, qsb, ksb, vsb, hf):
            """scores + softmax + weighted sum for 8 heads (DVE + ACT +
            Pool).  Halving lets the o-split and the first y chains start
            one half-attn earlier."""
            if hf == 0:
                sm_sb[t] = (
                    sm_pool.tile([P, H * G], F32, tag="sc", name="sc"),
                    sm_pool.tile([P, H * G], F32, tag="ex", name="ex"),
                    sm_pool.tile([P, H], F32, tag="dn", name="dn"),
                    sm_pool.tile([P, H], F32, tag="rc", name="rc"),
                    sm_pool.tile([P, H * G], F32, tag="pf", name="pf"),
                    sm_pool.tile([P, D], BF16, tag="junk", name="junk"),
                )
                obf_sb[t] = obf_pool.tile([P, HID], BF16, name="obf",
                                          tag="obf")
            sc, ex, dn, rc, pf, junk = sm_sb[t]
            obf = obf_sb[t]
            h0 = hf * HH
            cs = slice(h0 * G, (h0 + HH) * G)

            # raw scores sc[t,(h,g)] = <q_h, k_g>  (fused mult+reduce, DVE)
            for h in range(h0, h0 + HH):
                for g in range(G):
                    nc.vector.scalar_tensor_tensor(
                        junk[:],
                        qsb[:, h * D:(h + 1) * D],
                        1.0,
                        ksb[:, g * D:(g + 1) * D],
                        op0=mybir.AluOpType.mult,
                        op1=mybir.AluOpType.mult,
                        accum_out=sc[:, ds(h * G + g, 1)],
                    )

            # softmax over g; q,k carry x512 each -> exp scale /512^2
            nc.scalar.activation(
                ex[:, cs], sc[:, cs], mybir.ActivationFunctionType.Exp,
                scale=SCALE / (WS * WS))
            nc.vector.reduce_sum(
                dn[:, h0:h0 + HH],
                ex[:, cs].rearrange("p (h g) -> p h g", g=G),
                axis=mybir.AxisListType.X,
            )
            nc.vector.reciprocal(rc[:, h0:h0 + HH], dn[:, h0:h0 + HH])
            # pf = OS * ex / dn  (o shipped as 16*o for the fp8 split)
            nc.vector.scalar_tensor_tensor(
                pf[:, cs].rearrange("p (h g) -> p h g", g=G),
                ex[:, cs].rearrange("p (h g) -> p h g", g=G),
                OS,
                rc[:, h0:h0 + HH].unsqueeze(2).broadcast_to((P, HH, G)),
                op0=mybir.AluOpType.mult, op1=mybir.AluOpType.mult,
            )

            # o[t,(h,d)] = sum_g p[t,(h,g)] * v[t,(g,d)]  - batched per g on
            # the otherwise-idle GpSimd/Pool engine (4 mult + 3 accum ops of
            # [p, HH*D], broadcast views; DVE keeps only the score dots)
            tmp = wt_pool.tile([P, HW], BF16, tag="ta", name="ta")
            tm3 = tmp[:].rearrange("p (h d) -> p h d", d=D)
            ob3 = obf[:, hf * HW:(hf + 1) * HW] \
                .rearrange("p (h d) -> p h d", d=D)
            pfh = pf[:].rearrange("p (h g) -> p h g", g=G)[:, h0:h0 + HH, :]
            vv = lambda g: vsb[:, g * D:(g + 1) * D].unsqueeze(1) \
                .broadcast_to((P, HH, D))
            pp = lambda g: pfh[:, :, g:g + 1].broadcast_to((P, HH, D))
            nc.gpsimd.tensor_tensor(ob3, vv(0), pp(0),
                                    op=mybir.AluOpType.mult)
            for g in range(1, G):
                nc.gpsimd.tensor_tensor(tm3, vv(g), pp(g),
                                        op=mybir.AluOpType.mult)
                nc.gpsimd.tensor_tensor(ob3, ob3, tm3,
                                        op=mybir.AluOpType.add)

        def attn_middle(t, qsb, ksb, vsb):
            attn_half(t, qsb, ksb, vsb, 0)
            attn_half(t, qsb, ksb, vsb, 1)

        def split_half(t, hf):
            # O^T via the DMA xbar (keeps the PE matmul-only):
            # ot[p, o*128+tok] = obf[tok, o*128+p]; then split to fp8 hi/lo.
            if hf == 0:
                ot_sb[t] = ot_pool.tile([P, HID], BF16, name="ot", tag="ot")
                oth_sb[t] = oth_pool.tile([P, HID], F8, name="oth",
                                          tag="oth")
                otl_sb[t] = otl_pool.tile([P, HID], F8, name="otl",
                                          tag="otl")
            ot, oth, otl = ot_sb[t], oth_sb[t], otl_sb[t]
            hs = slice(hf * HW, (hf + 1) * HW)
            nc.sync.dma_start_transpose(
                ot[:, hs].rearrange("p (o t2) -> p o t2", t2=P),
                obf_sb[t][:, hs])
            # both split ops on Pool: ACT stays free of transpose-DMA
            # dependencies, so the PE's psum copies never queue behind them
            nc.gpsimd.tensor_copy(oth[:, hs], ot[:, hs])
            nc.gpsimd.tensor_sub(otl[:, hs], ot[:, hs], oth[:, hs])

        def transpose_split_o(t):
            split_half(t, 0)
            split_half(t, 1)
            obf_sb[t] = None
            ot_sb[t] = None

        def copy_qkv(which, c, ps, qsb, ksb, vsb):
            if which == "q":
                nc.scalar.copy(qsb[:, c * 512:(c + 1) * 512], ps[:])
            elif which == "k":
                nc.scalar.copy(ksb[:], ps[:])
            else:
                nc.scalar.mul(vsb[:], ps[:], 1.0 / WS)

        def emit_y_tile(tw):
            if tw == NTT - 1:
                # last tile: sequential chains, final ones narrow, so the
                # post-matmul drain holds only one short copy+DMA
                for c0, c1 in ((0, 512), (512, 1024), (1024, 1536),
                               (1536, 1792), (1792, 2048)):
                    yps = y_ps_pool.tile([P, c1 - c0], F32,
                                         name="yps", tag="yps")
                    emit_chain_pair([(yps, y_steps(tw, c0, c1))])
                    ysb = ysb_pool.tile([P, c1 - c0], BF16,
                                        name="ysb", tag="ysb")
                    nc.scalar.mul(ysb[:], yps[:], 1.0 / (WS * OS))
                    nc.sync.dma_start(
                        y[tw * P:(tw + 1) * P, c0:c1], ysb[:])
                oth_sb[tw] = None
                otl_sb[tw] = None
                return
            for sp in range(2):
                ypa = y_ps_pool.tile([P, 512], F32, name="yps", tag="yps")
                ypb = y_ps_pool.tile([P, 512], F32, name="yps", tag="yps")
                emit_chain_pair([
                    (ypa, y_steps(tw, 2 * sp * 512, (2 * sp + 1) * 512)),
                    (ypb, y_steps(tw, (2 * sp + 1) * 512, (2 * sp + 2) * 512)),
                ])
                for yps, s_ in ((ypa, 2 * sp), (ypb, 2 * sp + 1)):
                    ysb = ysb_pool.tile([P, 512], BF16, name="ysb", tag="ysb")
                    nc.scalar.mul(ysb[:], yps[:], 1.0 / (WS * OS))
                    nc.sync.dma_start(
                        y[tw * P:(tw + 1) * P, s_ * 512:(s_ + 1) * 512],
                        ysb[:])
            oth_sb[tw] = None
            otl_sb[tw] = None

        # ---- prologue: tiles 0-3 in three phases, each running 8 psum
        # chains (all banks) step-major so every weight chunk is consumed
        # by 4 reader tiles as it arrives ----
        NPT = 3
        pro_sb = {}
        for tt in range(NPT):
            pro_sb[tt] = (
                qsb_pool.tile([P, HID], BF16, tag="q", name="q"),
                qsb_pool.tile([P, KV], BF16, tag="k", name="k"),
                qsb_pool.tile([P, KV], BF16, tag="v", name="v"),
            )
        for chains in ([("q", 0), ("q", 1)], [("q", 2), ("q", 3)],
                       [("k", 0)], [("v", 0)]):
            pairs = []
            for ci, (which, c) in enumerate(chains):
                for tt in range(NPT):
                    pool = qkv_ps_pool if ci == 0 else y_ps_pool
                    tagname = "ps" if ci == 0 else "yps"
                    ps = pool.tile([P, 512], F32, name=tagname,
                                   tag=tagname)
                    pairs.append(
                        (ps, qkv_steps(tt, which, c),
                         (lambda w_, c_, t_: lambda psd: copy_qkv(
                             w_, c_, psd, *pro_sb[t_]))(which, c, tt)))
            emit_chain_pair(pairs)
        # attn + o-split interleaved per half-tile so each split sits
        # directly behind its weighted-sum in the Pool queue and the first
        # y chains start after only half an attn
        for tt in range(NPT):
            for hf in range(2):
                attn_half(tt, *pro_sb[tt], hf)
                split_half(tt, hf)
            obf_sb[tt] = None
            ot_sb[tt] = None
        load_xh(3)
        load_xl(3)
        load_xh(4)
        load_xl(4)

        # ---- steady state: QKV(t) | Wo(t-3 or t-4); the wo weight stream
        # is emitted at the first loop iteration, after every qkv weight and
        # the early x tiles, so the first y chains never wait on it ----
        for t in range(NPT, NTT + 4):
            if t <= NTT - 3:
                load_xh(t + 2)
                load_xl(t + 2)
            if t == NPT:
                # rest of wo, in Y(0)'s consumption order: its first chain
                # pair tails with wol cols 0:1024, then the second pair
                # opens on woh cols 1024:2048
                for j in range(4):
                    wchunk(wola, wol, 4 * j, 4 * (j + 1), 0, 1024, HID)
                for j in range(4):
                    wchunk(woha, woh, 4 * j, 4 * (j + 1), 1024, 2048, HID)
                for j in range(4):
                    wchunk(wola, wol, 4 * j, 4 * (j + 1), 1024, 2048, HID)

            if t < NTT:
                qsb = qsb_pool.tile([P, HID], BF16, tag="q")
                ksb = qsb_pool.tile([P, KV], BF16, tag="k")
                vsb = qsb_pool.tile([P, KV], BF16, tag="v")

                # 3 pair-interleaved chains; paired chains share the
                # stationary x pair per step (back-to-back reuse)
                for pi, (pa, pb) in enumerate(((("q", 0), ("k", 0)),
                                               (("q", 1), ("q", 2)),
                                               (("q", 3), ("v", 0)))):
                    pool = y_ps_pool if (t == NPT and pi == 2) \
                        else qkv_ps_pool
                    tg = "yps" if (t == NPT and pi == 2) else "ps"
                    psa = pool.tile([P, 512], F32, name=tg, tag=tg)
                    psb = pool.tile([P, 512], F32, name=tg, tag=tg)
                    emit_chain_pair([
                        (psa, qkv_steps(t, pa[0], pa[1])),
                        (psb, qkv_steps(t, pb[0], pb[1])),
                    ])
                    copy_qkv(pa[0], pa[1], psa, qsb, ksb, vsb)
                    copy_qkv(pb[0], pb[1], psb, qsb, ksb, vsb)

                attn_middle(t, qsb, ksb, vsb)

            # Wo matmuls + y out, lagged 5 tiles behind the QKV stream so
            # the first y chains never wait on the prologue's attn backlog
            # (4 tiles of DVE/Pool work drain at ~8us/iter) nor on the
            # wo weight DMA
            if t - 4 >= 0:
                emit_y_tile(t - 4)

            if NPT <= t < NTT:
                transpose_split_o(t)

    nc.compile()
    return nc


def _build_bias(has_bias: bool = True) -> bass.Bass:
    """Original (slower) path, kept for the biased case."""
    nc = bacc.Bacc("TRN2")
    x = nc.dram_tensor("x", [TPC, HID], BF16, kind="ExternalInput")
    wq = nc.dram_tensor("wq", [HID, HID], BF16, kind="ExternalInput")
    wk = nc.dram_tensor("wk", [HID, KV], BF16, kind="ExternalInput")
    wv = nc.dram_tensor("wv", [HID, KV], BF16, kind="ExternalInput")
    wo = nc.dram_tensor("wo", [HID, HID], BF16, kind="ExternalInput")
    if has_bias:
        bqkv = nc.dram_tensor("bqkv", [1, HID + 2 * KV], F32, kind="ExternalInput")
        bo = nc.dram_tensor("bo", [1, HID], F32, kind="ExternalInput")
    y = nc.dram_tensor("y", [TPC, HID], F32, kind="ExternalOutput")

    with tile.TileContext(nc) as tc, ExitStack() as ctx:
        const_pool = ctx.enter_context(tc.tile_pool(name="const", bufs=1))
        ident = const_pool.tile([P, P], BF16)
        make_identity(nc, ident[:])

        if has_bias:
            bias_qkv = const_pool.tile([P, HID + 2 * KV], F32)
            nc.sync.dma_start(bias_qkv[:], bqkv[0:1, :].broadcast_to((P, HID + 2 * KV)))
            bias_o = const_pool.tile([P, HID], F32)
            nc.sync.dma_start(bias_o[:], bo[0:1, :].broadcast_to((P, HID)))

        # O^T staging for the whole core: [o_block(16) x tokens(2048)] bf16
        ofm_pool = ctx.enter_context(tc.tile_pool(name="ofm", bufs=1))
        ofm = ofm_pool.tile([P, NI * TPC], BF16)

        kv_pool = ctx.enter_context(tc.tile_pool(name="wkv", bufs=1))
        wk_sb = []
        wv_sb = []
        for i in range(NI):
            wk_t = kv_pool.tile([P, KV], BF16, tag=f"wk{i}")
            nc.sync.dma_start(wk_t[:], wk[i * P:(i + 1) * P, :])
            wk_sb.append(wk_t)
            wv_t = kv_pool.tile([P, KV], BF16, tag=f"wv{i}")
            nc.sync.dma_start(wv_t[:], wv[i * P:(i + 1) * P, :])
            wv_sb.append(wv_t)

        pt_pool = ctx.enter_context(tc.tile_pool(name="pt", bufs=2, space="PSUM"))
        mm_pool = ctx.enter_context(tc.tile_pool(name="mm", bufs=3, space="PSUM"))

        # ---------------- Phase A: QKV projections + attention ----------------
        with tc.tile_pool(name="wqp", bufs=1) as wq_pool, \
             tc.tile_pool(name="xt", bufs=3) as xt_pool, \
             tc.tile_pool(name="xfm", bufs=1) as xfm_pool, \
             tc.tile_pool(name="qkv", bufs=1) as qkv_pool, \
             tc.tile_pool(name="attn", bufs=2) as attn_pool, \
             tc.tile_pool(name="oacc", bufs=1) as oacc_pool, \
             tc.tile_pool(name="obf", bufs=1) as obf_pool:
            wq_sb = []
            for i in range(NI):
                wq_t = wq_pool.tile([P, HID], BF16, tag=f"wq{i}")
                nc.sync.dma_start(wq_t[:], wq[i * P:(i + 1) * P, :])
                wq_sb.append(wq_t)

            for t in range(NTT):
                xt = xt_pool.tile([P, HID], BF16)
                nc.sync.dma_start(xt[:], x[t * P:(t + 1) * P, :])

                # transpose X tile to feature-major [i, t] (16 blocks of 128x128)
                xfm = xfm_pool.tile([P, HID], BF16)
                for j in range(4):
                    pt = pt_pool.tile([P, 512], BF16)
                    for k in range(4):
                        blk = 4 * j + k
                        nc.tensor.transpose(
                            pt[:, k * P:(k + 1) * P],
                            xt[:, blk * P:(blk + 1) * P],
                            ident[:],
                        )
                    nc.vector.tensor_copy(xfm[:, j * 512:(j + 1) * 512], pt[:])

                # QKV projections, token-major out: [t(128part), 3072]
                qkv = qkv_pool.tile([P, HID + 2 * KV], F32)
                for s in range(6):
                    ps = mm_pool.tile([P, 512], F32)
                    for i in range(NI):
                        if s < 4:
                            rhs = wq_sb[i][:, s * 512:(s + 1) * 512]
                        elif s == 4:
                            rhs = wk_sb[i][:]
                        else:
                            rhs = wv_sb[i][:]
                        nc.tensor.matmul(
                            ps[:], xfm[:, i * P:(i + 1) * P], rhs,
                            start=(i == 0), stop=(i == NI - 1),
                        )
                    if has_bias:
                        nc.vector.tensor_add(
                            qkv[:, s * 512:(s + 1) * 512], ps[:],
                            bias_qkv[:, s * 512:(s + 1) * 512],
                        )
                    else:
                        nc.vector.tensor_copy(qkv[:, s * 512:(s + 1) * 512], ps[:])

                # scores[t, h, g] = <q_h, k_g> * SCALE   (fused mult+reduce)
                sc = attn_pool.tile([P, H * G], F32, tag="sc")
                junk = attn_pool.tile([P, D], F32, tag="junk")
                for h in range(H):
                    for g in range(G):
                        nc.vector.scalar_tensor_tensor(
                            junk[:],
                            qkv[:, h * D:(h + 1) * D],
                            SCALE,
                            qkv[:, HID + g * D:HID + (g + 1) * D],
                            op0=mybir.AluOpType.mult,
                            op1=mybir.AluOpType.mult,
                            accum_out=sc[:, ds(h * G + g, 1)],
                        )

                # softmax over g (4); denominator folded into final scale
                ex = attn_pool.tile([P, H * G], F32, tag="ex")
                nc.scalar.activation(ex[:], sc[:], mybir.ActivationFunctionType.Exp)
                dn = attn_pool.tile([P, H], F32, tag="dn")
                nc.vector.reduce_sum(
                    dn[:], ex[:].rearrange("p (h g) -> p h g", g=G),
                    axis=mybir.AxisListType.X,
                )
                rc = attn_pool.tile([P, H], F32, tag="rc")
                nc.vector.reciprocal(rc[:], dn[:])

                # o[t, h*D+d] = (sum_g ex[t,h,g] * v[t, g*D+d]) * rc[t,h]
                acc = oacc_pool.tile([P, HID], F32, tag="acc")
                tmp = oacc_pool.tile([P, HID], F32, tag="tmp")
                obf = obf_pool.tile([P, HID], BF16)
                ab = [acc, tmp]
                for h in range(H):
                    hs = ds(h * D, D)
                    nc.vector.tensor_scalar_mul(
                        ab[0][:, hs],
                        qkv[:, HID + KV:HID + KV + D],
                        ex[:, ds(h * G, 1)],
                    )
                    for g in range(1, G):
                        nc.vector.scalar_tensor_tensor(
                            ab[g % 2][:, hs],
                            qkv[:, HID + KV + g * D:HID + KV + (g + 1) * D],
                            ex[:, ds(h * G + g, 1)],
                            ab[(g - 1) % 2][:, hs],
                            op0=mybir.AluOpType.mult,
                            op1=mybir.AluOpType.add,
                        )
                    nc.vector.tensor_scalar_mul(
                        obf[:, hs], ab[(G - 1) % 2][:, hs], rc[:, ds(h, 1)])

                # transpose O tile into ofm [o_block, token]
                for j in range(4):
                    pt = pt_pool.tile([P, 512], BF16)
                    for k in range(4):
                        blk = 4 * j + k
                        nc.tensor.transpose(
                            pt[:, k * P:(k + 1) * P],
                            obf[:, blk * P:(blk + 1) * P],
                            ident[:],
                        )
                    nc.vector.tensor_copy(
                        ofm[:].rearrange("p (o t) -> p o t", t=TPC)
                              [:, 4 * j:4 * j + 4, t * P:(t + 1) * P],
                        pt[:].rearrange("p (o t) -> p o t", t=P),
                    )

        # ---------------- Phase B: output projection ----------------
        with tc.tile_pool(name="wop", bufs=1) as wo_pool, \
             tc.tile_pool(name="yt", bufs=3) as yt_pool:
            wo_sb = []
            for i in range(NI):
                wo_t = wo_pool.tile([P, HID], BF16, tag=f"wo{i}")
                nc.sync.dma_start(wo_t[:], wo[i * P:(i + 1) * P, :])
                wo_sb.append(wo_t)

            for t in range(NTT):
                for s in range(4):
                    ps = mm_pool.tile([P, 512], F32)
                    for o in range(NI):
                        nc.tensor.matmul(
                            ps[:],
                            ofm[:, ds(o * TPC + t * P, P)],
                            wo_sb[o][:, s * 512:(s + 1) * 512],
                            start=(o == 0), stop=(o == NI - 1),
                        )
                    yt = yt_pool.tile([P, 512], F32)
                    if has_bias:
                        nc.vector.tensor_add(
                            yt[:], ps[:], bias_o[:, s * 512:(s + 1) * 512])
                    else:
                        nc.vector.tensor_copy(yt[:], ps[:])
                    nc.sync.dma_start(
                        y[t * P:(t + 1) * P, s * 512:(s + 1) * 512], yt[:])

    nc.compile()
    return nc


def _build(has_bias: bool) -> bass.Bass:
    return _build_bias(True) if has_bias else _build_fp8()


def kernel(hidden_states, Wq, bq, Wk, bk, Wv, bv, Wo, bo, _profile=None):
    has_bias = bool(np.any(bq) or np.any(bk) or np.any(bv) or np.any(bo))
    key = has_bias
    if key not in _cache:
        _cache[key] = _build(has_bias)
    nc = _cache[key]

    x_flat = np.ascontiguousarray(
        np.asarray(hidden_states, dtype=np.float32).reshape(NTOK, HID))

    in_maps = []
    if has_bias:
        bf = ml_dtypes.bfloat16
        xb = x_flat.astype(bf)
        wq_b = np.asarray(Wq, dtype=np.float32).astype(bf)
        wk_b = np.asarray(Wk, dtype=np.float32).astype(bf)
        wv_b = np.asarray(Wv, dtype=np.float32).astype(bf)
        wo_b = np.asarray(Wo, dtype=np.float32).astype(bf)
        for c in range(NCORES):
            m = {
                "x": np.ascontiguousarray(xb[c * TPC:(c + 1) * TPC]),
                "wq": wq_b, "wk": wk_b, "wv": wv_b, "wo": wo_b,
                "bqkv": np.concatenate([
                    np.asarray(bq, np.float32), np.asarray(bk, np.float32),
                    np.asarray(bv, np.float32)]).reshape(1, HID + 2 * KV),
                "bo": np.asarray(bo, np.float32).reshape(1, HID),
            }
            in_maps.append(m)
    else:
        e4 = ml_dtypes.float8_e4m3
        xh8 = x_flat.astype(e4)
        xl8 = (x_flat - xh8.astype(np.float32)).astype(e4)

        def wsplit(W):
            Wf = np.asarray(W, dtype=np.float32) * WS
            hi = Wf.astype(e4)
            lo = (Wf - hi.astype(np.float32)).astype(e4)
            return np.ascontiguousarray(hi), np.ascontiguousarray(lo)

        wq8 = np.ascontiguousarray(
            (np.asarray(Wq, np.float32) * WS).astype(e4))
        wk8 = np.ascontiguousarray(
            (np.asarray(Wk, np.float32) * WS).astype(e4))
        wvh8, wvl8 = wsplit(Wv)
        woh8, wol8 = wsplit(Wo)

        def pret(a):
            # host pre-transpose: row (t*128+p), col (i*128+tok) <- x[(t,tok),(i,p)]
            return np.ascontiguousarray(
                a.reshape(NTT, P, NI, P).transpose(0, 3, 2, 1).reshape(TPC, HID))

        for c in range(NCORES):
            m = {
                "xh": pret(xh8[c * TPC:(c + 1) * TPC]),
                "xl": pret(xl8[c * TPC:(c + 1) * TPC]),
                "wq": wq8, "wk": wk8,
                "wvh": wvh8, "wvl": wvl8,
                "woh": woh8, "wol": wol8,
            }
            in_maps.append(m)

    kwargs = dict(_profile) if _profile else {}
    kwargs.pop("result", None)
    res = run_bass_kernel_spmd(nc, in_maps, list(range(NCORES)), **kwargs)
    out = np.concatenate([r["y"] for r in res.results], axis=0)
    if _profile is not None:
        _profile["result"] = res
    return out.reshape(B, S, HID).astype(np.float32)


# revision 26
# speedup vs baseline: 1.0308x; 1.0020x over previous
"""Trainium2 Bass kernel for per-position grouped-query attention.

Reference computation (B=4, S=4096, HID=2048, H=16, G=4, D=128, KV=512):
    q = x @ Wq + bq ; k = x @ Wk + bk ; v = x @ Wv + bv
    scores[t,h,g] = <q[t,h,:], k[t,g,:]> / sqrt(D)     (same-position only)
    probs = softmax_g(scores)
    o[t,h,:] = sum_g probs[t,h,g] * v[t,g,:]
    y = o @ Wo + bo

Strategy: data-parallel over the 16384 flattened tokens -> 2048 tokens/core
on 8 cores, all weights replicated, no collectives.  The matmuls run as
fp8-e4m3 DoubleRow (2 contraction blocks per instruction, 0.5 cyc/row ->
4x bf16 MAC rate), with compensated splits to stay inside the 2e-2 gate:
  - x is shipped as an fp8 (hi, lo) pair: xl = fp8(x - fp8(x)).
  - Q/K projections: (xh + xl) @ fp8(512 W)  - 2 chain segments each; the
    remaining weight-quantization noise only reaches the output through the
    4-way softmax, measured ~1.1e-2 end to end.
  - V projection: xh@Wvh + xl@Wvh + xh@Wvl  (weights split hi/lo) - ~exact.
  - attention middle on DVE/ACT exactly as before (bf16 staging, ~0.1%).
  - O^T (bf16, via DMA-xbar transpose) is split on-chip into fp8 hi/lo
    (ACT cast + DVE subtract) and o @ Wo runs the same 3-chain compensated
    form.  Scale bookkeeping: weights x512, o x16, exp scale /512^2,
    y copy /8192.
Per tile the PE does 51200 cycles (vs 81920 bf16) -> ~341us across 16
tiles; weight DMA (15 MiB fp8) overlaps the 2-tile prologue like before.
"""

import os
import sys

import numpy as np

sys.path.insert(0, "/opt/trn_rl_repo")

import ml_dtypes  # noqa: E402
from contextlib import ExitStack  # noqa: E402

import concourse.bass as bass  # noqa: E402
import concourse.bacc as bacc  # noqa: E402
import concourse.mybir as mybir  # noqa: E402
import concourse.tile as tile  # noqa: E402
from concourse.bass import ds  # noqa: E402
from concourse.bass_utils import run_bass_kernel_spmd  # noqa: E402
from concourse.masks import make_identity  # noqa: E402

B, S, HID = 4, 4096, 2048
H, G = 16, 4
D = HID // H          # 128
KV = HID * G // H     # 512
NCORES = 8
NTOK = B * S          # 16384
TPC = NTOK // NCORES  # 2048 tokens per core
P = 128
NTT = TPC // P        # 16 token tiles per core
NI = HID // P         # 16 input-feature blocks
NSP = NI // 2         # 8 DoubleRow step-pairs over the contraction
SCALE = 1.0 / float(np.sqrt(D))
WS = 512.0            # weight fp8 scale
OS = 16.0             # o fp8 scale

BF16 = mybir.dt.bfloat16
F32 = mybir.dt.float32
F8 = mybir.dt.float8e4
DR = mybir.MatmulPerfMode.DoubleRow

_cache = {}


def _build_fp8() -> bass.Bass:
    """No-bias fast path: fp8 DoubleRow matmuls with compensated splits."""
    nc = bacc.Bacc("TRN2")
    # xh/xl: host-pretransposed per token tile: row (t*128+p), col (i*128+tok)
    # holds x[t*128+tok, i*128+p]  -> per tile a plain [128, 2048] slice whose
    # block i is the lhsT [feat-in-block, token] for the QKV matmuls.
    xh = nc.dram_tensor("xh", [TPC, HID], F8, kind="ExternalInput")
    xl = nc.dram_tensor("xl", [TPC, HID], F8, kind="ExternalInput")
    wq = nc.dram_tensor("wq", [HID, HID], F8, kind="ExternalInput")
    wk = nc.dram_tensor("wk", [HID, KV], F8, kind="ExternalInput")
    wvh = nc.dram_tensor("wvh", [HID, KV], F8, kind="ExternalInput")
    wvl = nc.dram_tensor("wvl", [HID, KV], F8, kind="ExternalInput")
    woh = nc.dram_tensor("woh", [HID, HID], F8, kind="ExternalInput")
    wol = nc.dram_tensor("wol", [HID, HID], F8, kind="ExternalInput")
    y = nc.dram_tensor("y", [TPC, HID], BF16, kind="ExternalOutput")

    with tile.TileContext(nc) as tc, ExitStack() as ctx:
        w_pool = ctx.enter_context(tc.tile_pool(name="w", bufs=1))
        xh_pool = ctx.enter_context(tc.tile_pool(name="xh", bufs=3))
        xl_pool = ctx.enter_context(tc.tile_pool(name="xl", bufs=3))
        ysb_pool = ctx.enter_context(tc.tile_pool(name="ysb", bufs=2))
        qkv_ps_pool = ctx.enter_context(
            tc.tile_pool(name="qkvps", bufs=4, space="PSUM"))
        y_ps_pool = ctx.enter_context(tc.tile_pool(name="yps", bufs=4, space="PSUM"))
        qsb_pool = ctx.enter_context(tc.tile_pool(name="qsb", bufs=4))
        sm_pool = ctx.enter_context(tc.tile_pool(name="sm", bufs=2))
        wt_pool = ctx.enter_context(tc.tile_pool(name="wt", bufs=1))
        obf_pool = ctx.enter_context(tc.tile_pool(name="obf", bufs=2))
        ot_pool = ctx.enter_context(tc.tile_pool(name="ot", bufs=1))
        oth_pool = ctx.enter_context(tc.tile_pool(name="oth", bufs=4))
        otl_pool = ctx.enter_context(tc.tile_pool(name="otl", bufs=4))

        xh_sb = [None] * NTT
        xl_sb = [None] * NTT
        sm_sb = [None] * NTT
        obf_sb = [None] * NTT
        ot_sb = [None] * NTT
        oth_sb = [None] * NTT
        otl_sb = [None] * NTT

        def load_xh(t):
            xh_sb[t] = xh_pool.tile([P, HID], F8, name="xh", tag="xh")
            nc.sync.dma_start(xh_sb[t][:], xh[t * P:(t + 1) * P, :])

        def load_xl(t):
            xl_sb[t] = xl_pool.tile([P, HID], F8, name="xl", tag="xl")
            nc.sync.dma_start(xl_sb[t][:], xl[t * P:(t + 1) * P, :])

        # DMA emission order is the serial-DMA schedule: weights arrive in
        # the prologue's consumption order (chunked by contraction block) so
        # the DMA-bound startup overlaps the first two tiles' compute.
        xh_sb[0] = xh_pool.tile([P, HID], F8, name="xh", tag="xh")
        nc.sync.dma_start(xh_sb[0][:, 0:2 * P], xh[0:P, 0:2 * P])
        nc.sync.dma_start(xh_sb[0][:, 2 * P:], xh[0:P, 2 * P:])
        load_xh(1)

        wqa = w_pool.tile([P, NI * HID], F8, tag="wqa", name="wqa")
        wka = w_pool.tile([P, NI * KV], F8, tag="wka", name="wka")
        wvha = w_pool.tile([P, NI * KV], F8, tag="wvha", name="wvha")
        wvla = w_pool.tile([P, NI * KV], F8, tag="wvla", name="wvla")
        woha = w_pool.tile([P, NI * HID], F8, tag="woha", name="woha")
        wola = w_pool.tile([P, NI * HID], F8, tag="wola", name="wola")

        def wchunk(dst_all, src, i0, i1, c0, c1, w_, eng=None):
            # weight blocks i0..i1-1, cols c0:c1, one DMA
            (eng or nc.sync).dma_start(
                dst_all[:, i0 * w_:i1 * w_]
                .rearrange("p (i c) -> p i c", c=w_)[:, :, c0:c1],
                src[i0 * P:i1 * P, c0:c1]
                .rearrange("(i p) c -> p i c", p=P),
            )

        # Prologue covers 4 tiles (phases A/B/C below): with fp8 DoubleRow
        # the PE retires weight bytes 4x faster than bf16, so only >=4
        # concurrent reader tiles keep the serial weight DMA stream off the
        # critical path.  2-block chunks match the chains' pair consumption;
        # xl0-3 land mid-stream (xl needed from chain step 8).
        PAIRS = [(2 * j, 2 * j + 2) for j in range(NSP)]
        load_xh(2)
        # qkv weights ride TWO dma queues (SP + the Pool queue, idle until
        # the first weighted-sums ~34us in) - chunks alternate so each
        # stream carries half the bytes in consumption order
        ENGS = (None, nc.gpsimd)
        # phase A weights: wq cols 0:1024
        wchunk(wqa, wq, 0, 2, 0, 1024, HID)
        wchunk(wqa, wq, 2, 4, 0, 1024, HID, nc.gpsimd)
        load_xl(0)
        load_xl(1)
        wchunk(wqa, wq, 4, 6, 0, 1024, HID)
        wchunk(wqa, wq, 6, 8, 0, 1024, HID, nc.gpsimd)
        load_xl(2)
        for n, (i0, i1) in enumerate(PAIRS[4:]):
            wchunk(wqa, wq, i0, i1, 0, 1024, HID, ENGS[n % 2])
        # phase B weights: wq cols 1024:2048
        for n, (i0, i1) in enumerate(PAIRS):
            wchunk(wqa, wq, i0, i1, 1024, 2048, HID, ENGS[n % 2])
        # phase C weights: all of wk (its phase runs first so tile 0's
        # scores start early), then wvh, then wvl (consumed only from chain
        # step 16 of the V chains)
        for n, (i0, i1) in enumerate(PAIRS):
            wchunk(wka, wk, i0, i1, 0, KV, KV, ENGS[n % 2])
        for n, (i0, i1) in enumerate(PAIRS):
            wchunk(wvha, wvh, i0, i1, 0, KV, KV, ENGS[n % 2])
        for n, (i0, i1) in enumerate(PAIRS):
            wchunk(wvla, wvl, i0, i1, 0, KV, KV, ENGS[n % 2])
        # first half of wo-hi rides the phase-C DMA slack (phase C PE time
        # well exceeds its own weight bytes)
        for j in range(4):
            wchunk(woha, woh, 4 * j, 4 * (j + 1), 0, 1024, HID)

        wq_r = wqa[:].rearrange("p (i c) -> p i c", c=HID)
        wk_r = wka[:].rearrange("p (i c) -> p i c", c=KV)
        wvh_r = wvha[:].rearrange("p (i c) -> p i c", c=KV)
        wvl_r = wvla[:].rearrange("p (i c) -> p i c", c=KV)
        woh_r = woha[:].rearrange("p (i c) -> p i c", c=HID)
        wol_r = wola[:].rearrange("p (i c) -> p i c", c=HID)

        def xpair(t, s, lo):
            src = xl_sb[t] if lo else xh_sb[t]
            return src[:, 2 * s * P:(2 * s + 2) * P].rearrange(
                "p (i t2) -> p i t2", t2=P)

        def qkv_steps(t, which, c=0):
            """DoubleRow (lhsT, rhs) step list for one 512-col psum chain."""
            if which == "q":
                segs = [(False, wq_r), (True, wq_r)]
                cs = slice(c * 512, (c + 1) * 512)
            elif which == "k":
                segs = [(False, wk_r), (True, wk_r)]
                cs = slice(0, KV)
            else:  # v
                segs = [(False, wvh_r), (True, wvh_r), (False, wvl_r)]
                cs = slice(0, KV)
            steps = []
            for lo, w_r in segs:
                for s in range(NSP):
                    steps.append((xpair(t, s, lo), w_r[:, 2 * s:2 * s + 2, cs]))
            return steps

        def opair(t, s, lo):
            src = otl_sb[t] if lo else oth_sb[t]
            return src[:, 2 * s * P:(2 * s + 2) * P].rearrange(
                "p (o t2) -> p o t2", t2=P)

        def y_steps(tw, c0, c1):
            # o-half-major over the woh segments (so the chain starts as
            # soon as the first o-split half lands); the wol segment stays
            # last - it is the final arrival in the weight DMA stream
            cs = slice(c0, c1)
            steps = []
            for half in range(2):
                for lo in (False, True):
                    w_r = woh_r
                    for s in range(half * NSP // 2, (half + 1) * NSP // 2):
                        steps.append(
                            (opair(tw, s, lo), w_r[:, 2 * s:2 * s + 2, cs]))
            for s in range(NSP):
                steps.append((opair(tw, s, False), wol_r[:, 2 * s:2 * s + 2, cs]))
            return steps

        def emit_chain_pair(pairs):
            """pairs: list of (psum, steps[, on_done]); interleave
            step-wise, firing on_done(ps) right after a chain's stop so its
            psum copy is emitted as early as possible."""
            n = max(len(p[1]) for p in pairs)
            for s in range(n):
                for p in pairs:
                    ps, st = p[0], p[1]
                    if s < len(st):
                        lhs, rhs = st[s]
                        nc.tensor.matmul(
                            ps[:], lhs, rhs,
                            start=(s == 0), stop=(s == len(st) - 1),
                            perf_mode=DR,
                        )
                        if s == len(st) - 1 and len(p) > 2:
                            p[2](ps)

        HH = H // 2
        HW = HH * D  # 1024, half of the o columns

        def attn_half(t, qsb, ksb, vsb, hf):
            """scores + softmax + weighted sum for 8 heads (DVE + ACT +
            Pool).  Halving lets the o-split and the first y chains start
            one half-attn earlier."""
            if hf == 0:
                sm_sb[t] = (
                    sm_pool.tile([P, H * G], F32, tag="sc", name="sc"),
                    sm_pool.tile([P, H * G], F32, tag="ex", name="ex"),
                    sm_pool.tile([P, H], F32, tag="dn", name="dn"),
                    sm_pool.tile([P, H], F32, tag="rc", name="rc"),
                    sm_pool.tile([P, H * G], F32, tag="pf", name="pf"),
                    sm_pool.tile([P, D], BF16, tag="junk", name="junk"),
                )
                obf_sb[t] = obf_pool.tile([P, HID], BF16, name="obf",
                                          tag="obf")
            sc, ex, dn, rc, pf, junk = sm_sb[t]
            obf = obf_sb[t]
            h0 = hf * HH
            cs = slice(h0 * G, (h0 + HH) * G)

            # raw scores sc[t,(h,g)] = <q_h, k_g>  (fused mult+reduce, DVE)
            for h in range(h0, h0 + HH):
                for g in range(G):
                    nc.vector.scalar_tensor_tensor(
                        junk[:],
                        qsb[:, h * D:(h + 1) * D],
                        1.0,
                        ksb[:, g * D:(g + 1) * D],
                        op0=mybir.AluOpType.mult,
                        op1=mybir.AluOpType.mult,
                        accum_out=sc[:, ds(h * G + g, 1)],
                    )

            # softmax over g; q,k carry x512 each -> exp scale /512^2
            nc.scalar.activation(
                ex[:, cs], sc[:, cs], mybir.ActivationFunctionType.Exp,
                scale=SCALE / (WS * WS))
            nc.vector.reduce_sum(
                dn[:, h0:h0 + HH],
                ex[:, cs].rearrange("p (h g) -> p h g", g=G),
                axis=mybir.AxisListType.X,
            )
            nc.vector.reciprocal(rc[:, h0:h0 + HH], dn[:, h0:h0 + HH])
            # pf = OS * ex / dn  (o shipped as 16*o for the fp8 split)
            nc.vector.scalar_tensor_tensor(
                pf[:, cs].rearrange("p (h g) -> p h g", g=G),
                ex[:, cs].rearrange("p (h g) -> p h g", g=G),
                OS,
                rc[:, h0:h0 + HH].unsqueeze(2).broadcast_to((P, HH, G)),
                op0=mybir.AluOpType.mult, op1=mybir.AluOpType.mult,
            )

            # o[t,(h,d)] = sum_g p[t,(h,g)] * v[t,(g,d)]  - batched per g on
            # the otherwise-idle GpSimd/Pool engine (4 mult + 3 accum ops of
            # [p, HH*D], broadcast views; DVE keeps only the score dots)
            tmp = wt_pool.tile([P, HW], BF16, tag="ta", name="ta")
            tm3 = tmp[:].rearrange("p (h d) -> p h d", d=D)
            ob3 = obf[:, hf * HW:(hf + 1) * HW] \
                .rearrange("p (h d) -> p h d", d=D)
            pfh = pf[:].rearrange("p (h g) -> p h g", g=G)[:, h0:h0 + HH, :]
            vv = lambda g: vsb[:, g * D:(g + 1) * D].unsqueeze(1) \
                .broadcast_to((P, HH, D))
            pp = lambda g: pfh[:, :, g:g + 1].broadcast_to((P, HH, D))
            nc.gpsimd.tensor_tensor(ob3, vv(0), pp(0),
                                    op=mybir.AluOpType.mult)
            for g in range(1, G):
                nc.gpsimd.tensor_tensor(tm3, vv(g), pp(g),
                                        op=mybir.AluOpType.mult)
                nc.gpsimd.tensor_tensor(ob3, ob3, tm3,
                                        op=mybir.AluOpType.add)

        def attn_middle(t, qsb, ksb, vsb):
            attn_half(t, qsb, ksb, vsb, 0)
            attn_half(t, qsb, ksb, vsb, 1)

        def split_half(t, hf):
            # O^T via the DMA xbar (keeps the PE matmul-only):
            # ot[p, o*128+tok] = obf[tok, o*128+p]; then split to fp8 hi/lo.
            if hf == 0:
                ot_sb[t] = ot_pool.tile([P, HID], BF16, name="ot", tag="ot")
                oth_sb[t] = oth_pool.tile([P, HID], F8, name="oth",
                                          tag="oth")
                otl_sb[t] = otl_pool.tile([P, HID], F8, name="otl",
                                          tag="otl")
            ot, oth, otl = ot_sb[t], oth_sb[t], otl_sb[t]
            hs = slice(hf * HW, (hf + 1) * HW)
            nc.sync.dma_start_transpose(
                ot[:, hs].rearrange("p (o t2) -> p o t2", t2=P),
                obf_sb[t][:, hs])
            # both split ops on Pool: ACT stays free of transpose-DMA
            # dependencies, so the PE's psum copies never queue behind them
            nc.gpsimd.tensor_copy(oth[:, hs], ot[:, hs])
            nc.gpsimd.tensor_sub(otl[:, hs], ot[:, hs], oth[:, hs])

        def transpose_split_o(t):
            split_half(t, 0)
            split_half(t, 1)
            obf_sb[t] = None
            ot_sb[t] = None

        def copy_qkv(which, c, ps, qsb, ksb, vsb):
            if which == "q":
                nc.scalar.copy(qsb[:, c * 512:(c + 1) * 512], ps[:])
            elif which == "k":
                nc.scalar.copy(ksb[:], ps[:])
            else:
                nc.scalar.mul(vsb[:], ps[:], 1.0 / WS)

        def emit_y_tile(tw):
            if tw == NTT - 1:
                # last tile: sequential chains, final ones narrow, so the
                # post-matmul drain holds only one short copy+DMA
                for c0, c1 in ((0, 512), (512, 1024), (1024, 1536),
                               (1536, 1792), (1792, 1920), (1920, 2048)):
                    yps = y_ps_pool.tile([P, c1 - c0], F32,
                                         name="yps", tag="yps")
                    emit_chain_pair([(yps, y_steps(tw, c0, c1))])
                    ysb = ysb_pool.tile([P, c1 - c0], BF16,
                                        name="ysb", tag="ysb")
                    nc.scalar.mul(ysb[:], yps[:], 1.0 / (WS * OS))
                    nc.sync.dma_start(
                        y[tw * P:(tw + 1) * P, c0:c1], ysb[:])
                oth_sb[tw] = None
                otl_sb[tw] = None
                return
            for sp in range(2):
                ypa = y_ps_pool.tile([P, 512], F32, name="yps", tag="yps")
                ypb = y_ps_pool.tile([P, 512], F32, name="yps", tag="yps")
                emit_chain_pair([
                    (ypa, y_steps(tw, 2 * sp * 512, (2 * sp + 1) * 512)),
                    (ypb, y_steps(tw, (2 * sp + 1) * 512, (2 * sp + 2) * 512)),
                ])
                for yps, s_ in ((ypa, 2 * sp), (ypb, 2 * sp + 1)):
                    ysb = ysb_pool.tile([P, 512], BF16, name="ysb", tag="ysb")
                    nc.scalar.mul(ysb[:], yps[:], 1.0 / (WS * OS))
                    nc.sync.dma_start(
                        y[tw * P:(tw + 1) * P, s_ * 512:(s_ + 1) * 512],
                        ysb[:])
            oth_sb[tw] = None
            otl_sb[tw] = None

        # ---- prologue: tiles 0-3 in three phases, each running 8 psum
        # chains (all banks) step-major so every weight chunk is consumed
        # by 4 reader tiles as it arrives ----
        NPT = 3
        pro_sb = {}
        for tt in range(NPT):
            pro_sb[tt] = (
                qsb_pool.tile([P, HID], BF16, tag="q", name="q"),
                qsb_pool.tile([P, KV], BF16, tag="k", name="k"),
                qsb_pool.tile([P, KV], BF16, tag="v", name="v"),
            )
        for chains in ([("q", 0), ("q", 1)], [("q", 2), ("q", 3)],
                       [("k", 0)], [("v", 0)]):
            pairs = []
            for ci, (which, c) in enumerate(chains):
                for tt in range(NPT):
                    pool = qkv_ps_pool if ci == 0 else y_ps_pool
                    tagname = "ps" if ci == 0 else "yps"
                    ps = pool.tile([P, 512], F32, name=tagname,
                                   tag=tagname)
                    pairs.append(
                        (ps, qkv_steps(tt, which, c),
                         (lambda w_, c_, t_: lambda psd: copy_qkv(
                             w_, c_, psd, *pro_sb[t_]))(which, c, tt)))
            emit_chain_pair(pairs)
        # attn + o-split interleaved per half-tile so each split sits
        # directly behind its weighted-sum in the Pool queue and the first
        # y chains start after only half an attn
        for tt in range(NPT):
            for hf in range(2):
                attn_half(tt, *pro_sb[tt], hf)
                split_half(tt, hf)
            obf_sb[tt] = None
            ot_sb[tt] = None
        load_xh(3)
        load_xl(3)
        load_xh(4)
        load_xl(4)

        # ---- steady state: QKV(t) | Wo(t-3 or t-4); the wo weight stream
        # is emitted at the first loop iteration, after every qkv weight and
        # the early x tiles, so the first y chains never wait on it ----
        for t in range(NPT, NTT + 4):
            if t <= NTT - 3:
                load_xh(t + 2)
                load_xl(t + 2)
            if t == NPT:
                # rest of wo, in Y(0)'s consumption order: its first chain
                # pair tails with wol cols 0:1024, then the second pair
                # opens on woh cols 1024:2048
                for j in range(4):
                    wchunk(wola, wol, 4 * j, 4 * (j + 1), 0, 1024, HID)
                for j in range(4):
                    wchunk(woha, woh, 4 * j, 4 * (j + 1), 1024, 2048, HID)
                for j in range(4):
                    wchunk(wola, wol, 4 * j, 4 * (j + 1), 1024, 2048, HID)

            if t < NTT:
                qsb = qsb_pool.tile([P, HID], BF16, tag="q")
                ksb = qsb_pool.tile([P, KV], BF16, tag="k")
                vsb = qsb_pool.tile([P, KV], BF16, tag="v")

                # 3 pair-interleaved chains; paired chains share the
                # stationary x pair per step (back-to-back reuse)
                for pi, (pa, pb) in enumerate(((("q", 0), ("k", 0)),
                                               (("q", 1), ("q", 2)),
                                               (("q", 3), ("v", 0)))):
                    pool = y_ps_pool if (t == NPT and pi == 2) \
                        else qkv_ps_pool
                    tg = "yps" if (t == NPT and pi == 2) else "ps"
                    psa = pool.tile([P, 512], F32, name=tg, tag=tg)
                    psb = pool.tile([P, 512], F32, name=tg, tag=tg)
                    emit_chain_pair([
                        (psa, qkv_steps(t, pa[0], pa[1])),
                        (psb, qkv_steps(t, pb[0], pb[1])),
                    ])
                    copy_qkv(pa[0], pa[1], psa, qsb, ksb, vsb)
                    copy_qkv(pb[0], pb[1], psb, qsb, ksb, vsb)

                attn_middle(t, qsb, ksb, vsb)

            # Wo matmuls + y out, lagged 5 tiles behind the QKV stream so
            # the first y chains never wait on the prologue's attn backlog
            # (4 tiles of DVE/Pool work drain at ~8us/iter) nor on the
            # wo weight DMA
            if t - 4 >= 0:
                emit_y_tile(t - 4)

            if NPT <= t < NTT:
                transpose_split_o(t)

    nc.compile()
    return nc


def _build_bias(has_bias: bool = True) -> bass.Bass:
    """Original (slower) path, kept for the biased case."""
    nc = bacc.Bacc("TRN2")
    x = nc.dram_tensor("x", [TPC, HID], BF16, kind="ExternalInput")
    wq = nc.dram_tensor("wq", [HID, HID], BF16, kind="ExternalInput")
    wk = nc.dram_tensor("wk", [HID, KV], BF16, kind="ExternalInput")
    wv = nc.dram_tensor("wv", [HID, KV], BF16, kind="ExternalInput")
    wo = nc.dram_tensor("wo", [HID, HID], BF16, kind="ExternalInput")
    if has_bias:
        bqkv = nc.dram_tensor("bqkv", [1, HID + 2 * KV], F32, kind="ExternalInput")
        bo = nc.dram_tensor("bo", [1, HID], F32, kind="ExternalInput")
    y = nc.dram_tensor("y", [TPC, HID], F32, kind="ExternalOutput")

    with tile.TileContext(nc) as tc, ExitStack() as ctx:
        const_pool = ctx.enter_context(tc.tile_pool(name="const", bufs=1))
        ident = const_pool.tile([P, P], BF16)
        make_identity(nc, ident[:])

        if has_bias:
            bias_qkv = const_pool.tile([P, HID + 2 * KV], F32)
            nc.sync.dma_start(bias_qkv[:], bqkv[0:1, :].broadcast_to((P, HID + 2 * KV)))
            bias_o = const_pool.tile([P, HID], F32)
            nc.sync.dma_start(bias_o[:], bo[0:1, :].broadcast_to((P, HID)))

        # O^T staging for the whole core: [o_block(16) x tokens(2048)] bf16
        ofm_pool = ctx.enter_context(tc.tile_pool(name="ofm", bufs=1))
        ofm = ofm_pool.tile([P, NI * TPC], BF16)

        kv_pool = ctx.enter_context(tc.tile_pool(name="wkv", bufs=1))
        wk_sb = []
        wv_sb = []
        for i in range(NI):
            wk_t = kv_pool.tile([P, KV], BF16, tag=f"wk{i}")
            nc.sync.dma_start(wk_t[:], wk[i * P:(i + 1) * P, :])
            wk_sb.append(wk_t)
            wv_t = kv_pool.tile([P, KV], BF16, tag=f"wv{i}")
            nc.sync.dma_start(wv_t[:], wv[i * P:(i + 1) * P, :])
            wv_sb.append(wv_t)

        pt_pool = ctx.enter_context(tc.tile_pool(name="pt", bufs=2, space="PSUM"))
        mm_pool = ctx.enter_context(tc.tile_pool(name="mm", bufs=3, space="PSUM"))

        # ---------------- Phase A: QKV projections + attention ----------------
        with tc.tile_pool(name="wqp", bufs=1) as wq_pool, \
             tc.tile_pool(name="xt", bufs=3) as xt_pool, \
             tc.tile_pool(name="xfm", bufs=1) as xfm_pool, \
             tc.tile_pool(name="qkv", bufs=1) as qkv_pool, \
             tc.tile_pool(name="attn", bufs=2) as attn_pool, \
             tc.tile_pool(name="oacc", bufs=1) as oacc_pool, \
             tc.tile_pool(name="obf", bufs=1) as obf_pool:
            wq_sb = []
            for i in range(NI):
                wq_t = wq_pool.tile([P, HID], BF16, tag=f"wq{i}")
                nc.sync.dma_start(wq_t[:], wq[i * P:(i + 1) * P, :])
                wq_sb.append(wq_t)

            for t in range(NTT):
                xt = xt_pool.tile([P, HID], BF16)
                nc.sync.dma_start(xt[:], x[t * P:(t + 1) * P, :])

                # transpose X tile to feature-major [i, t] (16 blocks of 128x128)
                xfm = xfm_pool.tile([P, HID], BF16)
                for j in range(4):
                    pt = pt_pool.tile([P, 512], BF16)
                    for k in range(4):
                        blk = 4 * j + k
                        nc.tensor.transpose(
                            pt[:, k * P:(k + 1) * P],
                            xt[:, blk * P:(blk + 1) * P],
                            ident[:],
                        )
                    nc.vector.tensor_copy(xfm[:, j * 512:(j + 1) * 512], pt[:])

                # QKV projections, token-major out: [t(128part), 3072]
                qkv = qkv_pool.tile([P, HID + 2 * KV], F32)
                for s in range(6):
                    ps = mm_pool.tile([P, 512], F32)
                    for i in range(NI):
                        if s < 4:
                            rhs = wq_sb[i][:, s * 512:(s + 1) * 512]
                        elif s == 4:
                            rhs = wk_sb[i][:]
                        else:
                            rhs = wv_sb[i][:]
                        nc.tensor.matmul(
                            ps[:], xfm[:, i * P:(i + 1) * P], rhs,
                            start=(i == 0), stop=(i == NI - 1),
                        )
                    if has_bias:
                        nc.vector.tensor_add(
                            qkv[:, s * 512:(s + 1) * 512], ps[:],
                            bias_qkv[:, s * 512:(s + 1) * 512],
                        )
                    else:
                        nc.vector.tensor_copy(qkv[:, s * 512:(s + 1) * 512], ps[:])

                # scores[t, h, g] = <q_h, k_g> * SCALE   (fused mult+reduce)
                sc = attn_pool.tile([P, H * G], F32, tag="sc")
                junk = attn_pool.tile([P, D], F32, tag="junk")
                for h in range(H):
                    for g in range(G):
                        nc.vector.scalar_tensor_tensor(
                            junk[:],
                            qkv[:, h * D:(h + 1) * D],
                            SCALE,
                            qkv[:, HID + g * D:HID + (g + 1) * D],
                            op0=mybir.AluOpType.mult,
                            op1=mybir.AluOpType.mult,
                            accum_out=sc[:, ds(h * G + g, 1)],
                        )

                # softmax over g (4); denominator folded into final scale
                ex = attn_pool.tile([P, H * G], F32, tag="ex")
                nc.scalar.activation(ex[:], sc[:], mybir.ActivationFunctionType.Exp)
                dn = attn_pool.tile([P, H], F32, tag="dn")
                nc.vector.reduce_sum(
                    dn[:], ex[:].rearrange("p (h g) -> p h g", g=G),
                    axis=mybir.AxisListType.X,
                )
                rc = attn_pool.tile([P, H], F32, tag="rc")
                nc.vector.reciprocal(rc[:], dn[:])

                # o[t, h*D+d] = (sum_g ex[t,h,g] * v[t, g*D+d]) * rc[t,h]
                acc = oacc_pool.tile([P, HID], F32, tag="acc")
                tmp = oacc_pool.tile([P, HID], F32, tag="tmp")
                obf = obf_pool.tile([P, HID], BF16)
                ab = [acc, tmp]
                for h in range(H):
                    hs = ds(h * D, D)
                    nc.vector.tensor_scalar_mul(
                        ab[0][:, hs],
                        qkv[:, HID + KV:HID + KV + D],
                        ex[:, ds(h * G, 1)],
                    )
                    for g in range(1, G):
                        nc.vector.scalar_tensor_tensor(
                            ab[g % 2][:, hs],
                            qkv[:, HID + KV + g * D:HID + KV + (g + 1) * D],
                            ex[:, ds(h * G + g, 1)],
                            ab[(g - 1) % 2][:, hs],
                            op0=mybir.AluOpType.mult,
                            op1=mybir.AluOpType.add,
                        )
                    nc.vector.tensor_scalar_mul(
                        obf[:, hs], ab[(G - 1) % 2][:, hs], rc[:, ds(h, 1)])

                # transpose O tile into ofm [o_block, token]
                for j in range(4):
                    pt = pt_pool.tile([P, 512], BF16)
                    for k in range(4):
                        blk = 4 * j + k
                        nc.tensor.transpose(
                            pt[:, k * P:(k + 1) * P],
                            obf[:, blk * P:(blk + 1) * P],
                            ident[:],
                        )
                    nc.vector.tensor_copy(
                        ofm[:].rearrange("p (o t) -> p o t", t=TPC)
                              [:, 4 * j:4 * j + 4, t * P:(t + 1) * P],
                        pt[:].rearrange("p (o t) -> p o t", t=P),
                    )

        # ---------------- Phase B: output projection ----------------
        with tc.tile_pool(name="wop", bufs=1) as wo_pool, \
             tc.tile_pool(name="yt", bufs=3) as yt_pool:
            wo_sb = []
            for i in range(NI):
                wo_t = wo_pool.tile([P, HID], BF16, tag=f"wo{i}")
                nc.sync.dma_start(wo_t[:], wo[i * P:(i + 1) * P, :])
                wo_sb.append(wo_t)

            for t in range(NTT):
                for s in range(4):
                    ps = mm_pool.tile([P, 512], F32)
                    for o in range(NI):
                        nc.tensor.matmul(
                            ps[:],
                            ofm[:, ds(o * TPC + t * P, P)],
                            wo_sb[o][:, s * 512:(s + 1) * 512],
                            start=(o == 0), stop=(o == NI - 1),
                        )
                    yt = yt_pool.tile([P, 512], F32)
                    if has_bias:
                        nc.vector.tensor_add(
                            yt[:], ps[:], bias_o[:, s * 512:(s + 1) * 512])
                    else:
                        nc.vector.tensor_copy(yt[:], ps[:])
                    nc.sync.dma_start(
                        y[t * P:(t + 1) * P, s * 512:(s + 1) * 512], yt[:])

    nc.compile()
    return nc


def _build(has_bias: bool) -> bass.Bass:
    return _build_bias(True) if has_bias else _build_fp8()


def kernel(hidden_states, Wq, bq, Wk, bk, Wv, bv, Wo, bo, _profile=None):
    has_bias = bool(np.any(bq) or np.any(bk) or np.any(bv) or np.any(bo))
    key = has_bias
    if key not in _cache:
        _cache[key] = _build(has_bias)
    nc = _cache[key]

    x_flat = np.ascontiguousarray(
        np.asarray(hidden_states, dtype=np.float32).reshape(NTOK, HID))

    in_maps = []
    if has_bias:
        bf = ml_dtypes.bfloat16
        xb = x_flat.astype(bf)
        wq_b = np.asarray(Wq, dtype=np.float32).astype(bf)
        wk_b = np.asarray(Wk, dtype=np.float32).astype(bf)
        wv_b = np.asarray(Wv, dtype=np.float32).astype(bf)
        wo_b = np.asarray(Wo, dtype=np.float32).astype(bf)
        for c in range(NCORES):
            m = {
                "x": np.ascontiguousarray(xb[c * TPC:(c + 1) * TPC]),
                "wq": wq_b, "wk": wk_b, "wv": wv_b, "wo": wo_b,
                "bqkv": np.concatenate([
                    np.asarray(bq, np.float32), np.asarray(bk, np.float32),
                    np.asarray(bv, np.float32)]).reshape(1, HID + 2 * KV),
                "bo": np.asarray(bo, np.float32).reshape(1, HID),
            }
            in_maps.append(m)
    else:
        e4 = ml_dtypes.float8_e4m3
        xh8 = x_flat.astype(e4)
        xl8 = (x_flat - xh8.astype(np.float32)).astype(e4)

        def wsplit(W):
            Wf = np.asarray(W, dtype=np.float32) * WS
            hi = Wf.astype(e4)
            lo = (Wf - hi.astype(np.float32)).astype(e4)
            return np.ascontiguousarray(hi), np.ascontiguousarray(lo)

        wq8 = np.ascontiguousarray(
            (np.asarray(Wq, np.float32) * WS).astype(e4))
        wk8 = np.ascontiguousarray(
            (np.asarray(Wk, np.float32) * WS).astype(e4))
        wvh8, wvl8 = wsplit(Wv)
        woh8, wol8 = wsplit(Wo)

        def pret(a):
            # host pre-transpose: row (t*128+p), col (i*128+tok) <- x[(t,tok),(i,p)]
            return np.ascontiguousarray(
                a.reshape(NTT, P, NI, P).transpose(0, 3, 2, 1).reshape(TPC, HID))

        for c in range(NCORES):
            m = {
                "xh": pret(xh8[c * TPC:(c + 1) * TPC]),
                "xl": pret(xl8[c * TPC:(c + 1) * TPC]),
                "wq": wq8, "wk": wk8,
                "wvh": wvh8, "wvl": wvl8,
                "woh": woh8, "wol": wol8,
            }
            in_maps.append(m)

    kwargs = dict(_profile) if _profile else {}
    kwargs.pop("result", None)
    res = run_bass_kernel_spmd(nc, in_maps, list(range(NCORES)), **kwargs)
    out = np.concatenate([r["y"] for r in res.results], axis=0)
    if _profile is not None:
        _profile["result"] = res
    return out.reshape(B, S, HID).astype(np.float32)


# revision 27
# speedup vs baseline: 1.0316x; 1.0007x over previous
"""Trainium2 Bass kernel for per-position grouped-query attention.

Reference computation (B=4, S=4096, HID=2048, H=16, G=4, D=128, KV=512):
    q = x @ Wq + bq ; k = x @ Wk + bk ; v = x @ Wv + bv
    scores[t,h,g] = <q[t,h,:], k[t,g,:]> / sqrt(D)     (same-position only)
    probs = softmax_g(scores)
    o[t,h,:] = sum_g probs[t,h,g] * v[t,g,:]
    y = o @ Wo + bo

Strategy: data-parallel over the 16384 flattened tokens -> 2048 tokens/core
on 8 cores, all weights replicated, no collectives.  The matmuls run as
fp8-e4m3 DoubleRow (2 contraction blocks per instruction, 0.5 cyc/row ->
4x bf16 MAC rate), with compensated splits to stay inside the 2e-2 gate:
  - x is shipped as an fp8 (hi, lo) pair: xl = fp8(x - fp8(x)).
  - Q/K projections: (xh + xl) @ fp8(512 W)  - 2 chain segments each; the
    remaining weight-quantization noise only reaches the output through the
    4-way softmax, measured ~1.1e-2 end to end.
  - V projection: xh@Wvh + xl@Wvh + xh@Wvl  (weights split hi/lo) - ~exact.
  - attention middle on DVE/ACT exactly as before (bf16 staging, ~0.1%).
  - O^T (bf16, via DMA-xbar transpose) is split on-chip into fp8 hi/lo
    (ACT cast + DVE subtract) and o @ Wo runs the same 3-chain compensated
    form.  Scale bookkeeping: weights x512, o x16, exp scale /512^2,
    y copy /8192.
Per tile the PE does 51200 cycles (vs 81920 bf16) -> ~341us across 16
tiles; weight DMA (15 MiB fp8) overlaps the 2-tile prologue like before.
"""

import os
import sys

import numpy as np

sys.path.insert(0, "/opt/trn_rl_repo")

import ml_dtypes  # noqa: E402
from contextlib import ExitStack  # noqa: E402

import concourse.bass as bass  # noqa: E402
import concourse.bacc as bacc  # noqa: E402
import concourse.mybir as mybir  # noqa: E402
import concourse.tile as tile  # noqa: E402
from concourse.bass import ds  # noqa: E402
from concourse.bass_utils import run_bass_kernel_spmd  # noqa: E402
from concourse.masks import make_identity  # noqa: E402

B, S, HID = 4, 4096, 2048
H, G = 16, 4
D = HID // H          # 128
KV = HID * G // H     # 512
NCORES = 8
NTOK = B * S          # 16384
TPC = NTOK // NCORES  # 2048 tokens per core
P = 128
NTT = TPC // P        # 16 token tiles per core
NI = HID // P         # 16 input-feature blocks
NSP = NI // 2         # 8 DoubleRow step-pairs over the contraction
SCALE = 1.0 / float(np.sqrt(D))
WS = 512.0            # weight fp8 scale
OS = 16.0             # o fp8 scale

BF16 = mybir.dt.bfloat16
F32 = mybir.dt.float32
F8 = mybir.dt.float8e4
DR = mybir.MatmulPerfMode.DoubleRow

_cache = {}


def _build_fp8() -> bass.Bass:
    """No-bias fast path: fp8 DoubleRow matmuls with compensated splits."""
    nc = bacc.Bacc("TRN2")
    # xh/xl: host-pretransposed per token tile: row (t*128+p), col (i*128+tok)
    # holds x[t*128+tok, i*128+p]  -> per tile a plain [128, 2048] slice whose
    # block i is the lhsT [feat-in-block, token] for the QKV matmuls.
    xh = nc.dram_tensor("xh", [TPC, HID], F8, kind="ExternalInput")
    xl = nc.dram_tensor("xl", [TPC, HID], F8, kind="ExternalInput")
    wq = nc.dram_tensor("wq", [HID, HID], F8, kind="ExternalInput")
    wk = nc.dram_tensor("wk", [HID, KV], F8, kind="ExternalInput")
    wvh = nc.dram_tensor("wvh", [HID, KV], F8, kind="ExternalInput")
    wvl = nc.dram_tensor("wvl", [HID, KV], F8, kind="ExternalInput")
    woh = nc.dram_tensor("woh", [HID, HID], F8, kind="ExternalInput")
    wol = nc.dram_tensor("wol", [HID, HID], F8, kind="ExternalInput")
    y = nc.dram_tensor("y", [TPC, HID], BF16, kind="ExternalOutput")

    with tile.TileContext(nc) as tc, ExitStack() as ctx:
        w_pool = ctx.enter_context(tc.tile_pool(name="w", bufs=1))
        xh_pool = ctx.enter_context(tc.tile_pool(name="xh", bufs=3))
        xl_pool = ctx.enter_context(tc.tile_pool(name="xl", bufs=3))
        ysb_pool = ctx.enter_context(tc.tile_pool(name="ysb", bufs=2))
        qkv_ps_pool = ctx.enter_context(
            tc.tile_pool(name="qkvps", bufs=4, space="PSUM"))
        y_ps_pool = ctx.enter_context(tc.tile_pool(name="yps", bufs=4, space="PSUM"))
        qsb_pool = ctx.enter_context(tc.tile_pool(name="qsb", bufs=4))
        sm_pool = ctx.enter_context(tc.tile_pool(name="sm", bufs=2))
        wt_pool = ctx.enter_context(tc.tile_pool(name="wt", bufs=1))
        obf_pool = ctx.enter_context(tc.tile_pool(name="obf", bufs=2))
        ot_pool = ctx.enter_context(tc.tile_pool(name="ot", bufs=1))
        oth_pool = ctx.enter_context(tc.tile_pool(name="oth", bufs=4))
        otl_pool = ctx.enter_context(tc.tile_pool(name="otl", bufs=4))

        xh_sb = [None] * NTT
        xl_sb = [None] * NTT
        sm_sb = [None] * NTT
        obf_sb = [None] * NTT
        ot_sb = [None] * NTT
        oth_sb = [None] * NTT
        otl_sb = [None] * NTT

        def load_xh(t):
            xh_sb[t] = xh_pool.tile([P, HID], F8, name="xh", tag="xh")
            nc.sync.dma_start(xh_sb[t][:], xh[t * P:(t + 1) * P, :])

        def load_xl(t):
            xl_sb[t] = xl_pool.tile([P, HID], F8, name="xl", tag="xl")
            nc.sync.dma_start(xl_sb[t][:], xl[t * P:(t + 1) * P, :])

        # DMA emission order is the serial-DMA schedule: weights arrive in
        # the prologue's consumption order (chunked by contraction block) so
        # the DMA-bound startup overlaps the first two tiles' compute.
        xh_sb[0] = xh_pool.tile([P, HID], F8, name="xh", tag="xh")
        xh_sb[1] = xh_pool.tile([P, HID], F8, name="xh", tag="xh")
        xh_sb[2] = xh_pool.tile([P, HID], F8, name="xh", tag="xh")
        nc.sync.dma_start(xh_sb[0][:, 0:4 * P], xh[0:P, 0:4 * P])
        nc.gpsimd.dma_start(xh_sb[1][:, 0:4 * P], xh[P:2 * P, 0:4 * P])
        nc.sync.dma_start(xh_sb[2][:, 0:4 * P], xh[2 * P:3 * P, 0:4 * P])

        wqa = w_pool.tile([P, NI * HID], F8, tag="wqa", name="wqa")
        wka = w_pool.tile([P, NI * KV], F8, tag="wka", name="wka")
        wvha = w_pool.tile([P, NI * KV], F8, tag="wvha", name="wvha")
        wvla = w_pool.tile([P, NI * KV], F8, tag="wvla", name="wvla")
        woha = w_pool.tile([P, NI * HID], F8, tag="woha", name="woha")
        wola = w_pool.tile([P, NI * HID], F8, tag="wola", name="wola")

        def wchunk(dst_all, src, i0, i1, c0, c1, w_, eng=None):
            # weight blocks i0..i1-1, cols c0:c1, one DMA
            (eng or nc.sync).dma_start(
                dst_all[:, i0 * w_:i1 * w_]
                .rearrange("p (i c) -> p i c", c=w_)[:, :, c0:c1],
                src[i0 * P:i1 * P, c0:c1]
                .rearrange("(i p) c -> p i c", p=P),
            )

        # Prologue covers 4 tiles (phases A/B/C below): with fp8 DoubleRow
        # the PE retires weight bytes 4x faster than bf16, so only >=4
        # concurrent reader tiles keep the serial weight DMA stream off the
        # critical path.  2-block chunks match the chains' pair consumption;
        # xl0-3 land mid-stream (xl needed from chain step 8).
        PAIRS = [(2 * j, 2 * j + 2) for j in range(NSP)]
        # qkv weights ride TWO dma queues (SP + the Pool queue, idle until
        # the first weighted-sums ~34us in) - chunks alternate so each
        # stream carries half the bytes in consumption order
        ENGS = (None, nc.gpsimd)
        # phase A weights: wq cols 0:1024
        wchunk(wqa, wq, 0, 2, 0, 1024, HID)
        wchunk(wqa, wq, 2, 4, 0, 1024, HID, nc.gpsimd)
        nc.sync.dma_start(xh_sb[0][:, 4 * P:], xh[0:P, 4 * P:])
        nc.gpsimd.dma_start(xh_sb[1][:, 4 * P:], xh[P:2 * P, 4 * P:])
        nc.sync.dma_start(xh_sb[2][:, 4 * P:], xh[2 * P:3 * P, 4 * P:])
        load_xl(0)
        wchunk(wqa, wq, 4, 6, 0, 1024, HID)
        wchunk(wqa, wq, 6, 8, 0, 1024, HID, nc.gpsimd)
        load_xl(1)
        load_xl(2)
        for n, (i0, i1) in enumerate(PAIRS[4:]):
            wchunk(wqa, wq, i0, i1, 0, 1024, HID, ENGS[n % 2])
        # phase B weights: wq cols 1024:2048
        for n, (i0, i1) in enumerate(PAIRS):
            wchunk(wqa, wq, i0, i1, 1024, 2048, HID, ENGS[n % 2])
        # phase C weights: all of wk (its phase runs first so tile 0's
        # scores start early), then wvh, then wvl (consumed only from chain
        # step 16 of the V chains)
        for n, (i0, i1) in enumerate(PAIRS):
            wchunk(wka, wk, i0, i1, 0, KV, KV, ENGS[n % 2])
        for n, (i0, i1) in enumerate(PAIRS):
            wchunk(wvha, wvh, i0, i1, 0, KV, KV, ENGS[n % 2])
        for n, (i0, i1) in enumerate(PAIRS):
            wchunk(wvla, wvl, i0, i1, 0, KV, KV, ENGS[n % 2])
        # first half of wo-hi rides the phase-C DMA slack (phase C PE time
        # well exceeds its own weight bytes)
        for j in range(4):
            wchunk(woha, woh, 4 * j, 4 * (j + 1), 0, 1024, HID)

        wq_r = wqa[:].rearrange("p (i c) -> p i c", c=HID)
        wk_r = wka[:].rearrange("p (i c) -> p i c", c=KV)
        wvh_r = wvha[:].rearrange("p (i c) -> p i c", c=KV)
        wvl_r = wvla[:].rearrange("p (i c) -> p i c", c=KV)
        woh_r = woha[:].rearrange("p (i c) -> p i c", c=HID)
        wol_r = wola[:].rearrange("p (i c) -> p i c", c=HID)

        def xpair(t, s, lo):
            src = xl_sb[t] if lo else xh_sb[t]
            return src[:, 2 * s * P:(2 * s + 2) * P].rearrange(
                "p (i t2) -> p i t2", t2=P)

        def qkv_steps(t, which, c=0):
            """DoubleRow (lhsT, rhs) step list for one 512-col psum chain."""
            if which == "q":
                segs = [(False, wq_r), (True, wq_r)]
                cs = slice(c * 512, (c + 1) * 512)
            elif which == "k":
                segs = [(False, wk_r), (True, wk_r)]
                cs = slice(0, KV)
            else:  # v
                segs = [(False, wvh_r), (True, wvh_r), (False, wvl_r)]
                cs = slice(0, KV)
            steps = []
            for lo, w_r in segs:
                for s in range(NSP):
                    steps.append((xpair(t, s, lo), w_r[:, 2 * s:2 * s + 2, cs]))
            return steps

        def opair(t, s, lo):
            src = otl_sb[t] if lo else oth_sb[t]
            return src[:, 2 * s * P:(2 * s + 2) * P].rearrange(
                "p (o t2) -> p o t2", t2=P)

        def y_steps(tw, c0, c1):
            # o-half-major over the woh segments (so the chain starts as
            # soon as the first o-split half lands); the wol segment stays
            # last - it is the final arrival in the weight DMA stream
            cs = slice(c0, c1)
            steps = []
            for half in range(2):
                for lo in (False, True):
                    w_r = woh_r
                    for s in range(half * NSP // 2, (half + 1) * NSP // 2):
                        steps.append(
                            (opair(tw, s, lo), w_r[:, 2 * s:2 * s + 2, cs]))
            for s in range(NSP):
                steps.append((opair(tw, s, False), wol_r[:, 2 * s:2 * s + 2, cs]))
            return steps

        def emit_chain_pair(pairs):
            """pairs: list of (psum, steps[, on_done]); interleave
            step-wise, firing on_done(ps) right after a chain's stop so its
            psum copy is emitted as early as possible."""
            n = max(len(p[1]) for p in pairs)
            for s in range(n):
                for p in pairs:
                    ps, st = p[0], p[1]
                    if s < len(st):
                        lhs, rhs = st[s]
                        nc.tensor.matmul(
                            ps[:], lhs, rhs,
                            start=(s == 0), stop=(s == len(st) - 1),
                            perf_mode=DR,
                        )
                        if s == len(st) - 1 and len(p) > 2:
                            p[2](ps)

        HH = H // 2
        HW = HH * D  # 1024, half of the o columns

        def attn_half(t, qsb, ksb, vsb, hf):
            """scores + softmax + weighted sum for 8 heads (DVE + ACT +
            Pool).  Halving lets the o-split and the first y chains start
            one half-attn earlier."""
            if hf == 0:
                sm_sb[t] = (
                    sm_pool.tile([P, H * G], F32, tag="sc", name="sc"),
                    sm_pool.tile([P, H * G], F32, tag="ex", name="ex"),
                    sm_pool.tile([P, H], F32, tag="dn", name="dn"),
                    sm_pool.tile([P, H], F32, tag="rc", name="rc"),
                    sm_pool.tile([P, H * G], F32, tag="pf", name="pf"),
                    sm_pool.tile([P, D], BF16, tag="junk", name="junk"),
                )
                obf_sb[t] = obf_pool.tile([P, HID], BF16, name="obf",
                                          tag="obf")
            sc, ex, dn, rc, pf, junk = sm_sb[t]
            obf = obf_sb[t]
            h0 = hf * HH
            cs = slice(h0 * G, (h0 + HH) * G)

            # raw scores sc[t,(h,g)] = <q_h, k_g>  (fused mult+reduce, DVE)
            for h in range(h0, h0 + HH):
                for g in range(G):
                    nc.vector.scalar_tensor_tensor(
                        junk[:],
                        qsb[:, h * D:(h + 1) * D],
                        1.0,
                        ksb[:, g * D:(g + 1) * D],
                        op0=mybir.AluOpType.mult,
                        op1=mybir.AluOpType.mult,
                        accum_out=sc[:, ds(h * G + g, 1)],
                    )

            # softmax over g; q,k carry x512 each -> exp scale /512^2
            nc.scalar.activation(
                ex[:, cs], sc[:, cs], mybir.ActivationFunctionType.Exp,
                scale=SCALE / (WS * WS))
            nc.vector.reduce_sum(
                dn[:, h0:h0 + HH],
                ex[:, cs].rearrange("p (h g) -> p h g", g=G),
                axis=mybir.AxisListType.X,
            )
            nc.vector.reciprocal(rc[:, h0:h0 + HH], dn[:, h0:h0 + HH])
            # pf = OS * ex / dn  (o shipped as 16*o for the fp8 split)
            nc.vector.scalar_tensor_tensor(
                pf[:, cs].rearrange("p (h g) -> p h g", g=G),
                ex[:, cs].rearrange("p (h g) -> p h g", g=G),
                OS,
                rc[:, h0:h0 + HH].unsqueeze(2).broadcast_to((P, HH, G)),
                op0=mybir.AluOpType.mult, op1=mybir.AluOpType.mult,
            )

            # o[t,(h,d)] = sum_g p[t,(h,g)] * v[t,(g,d)]  - batched per g on
            # the otherwise-idle GpSimd/Pool engine (4 mult + 3 accum ops of
            # [p, HH*D], broadcast views; DVE keeps only the score dots)
            tmp = wt_pool.tile([P, HW], BF16, tag="ta", name="ta")
            tm3 = tmp[:].rearrange("p (h d) -> p h d", d=D)
            ob3 = obf[:, hf * HW:(hf + 1) * HW] \
                .rearrange("p (h d) -> p h d", d=D)
            pfh = pf[:].rearrange("p (h g) -> p h g", g=G)[:, h0:h0 + HH, :]
            vv = lambda g: vsb[:, g * D:(g + 1) * D].unsqueeze(1) \
                .broadcast_to((P, HH, D))
            pp = lambda g: pfh[:, :, g:g + 1].broadcast_to((P, HH, D))
            nc.gpsimd.tensor_tensor(ob3, vv(0), pp(0),
                                    op=mybir.AluOpType.mult)
            for g in range(1, G):
                nc.gpsimd.tensor_tensor(tm3, vv(g), pp(g),
                                        op=mybir.AluOpType.mult)
                nc.gpsimd.tensor_tensor(ob3, ob3, tm3,
                                        op=mybir.AluOpType.add)

        def attn_middle(t, qsb, ksb, vsb):
            attn_half(t, qsb, ksb, vsb, 0)
            attn_half(t, qsb, ksb, vsb, 1)

        def split_half(t, hf):
            # O^T via the DMA xbar (keeps the PE matmul-only):
            # ot[p, o*128+tok] = obf[tok, o*128+p]; then split to fp8 hi/lo.
            if hf == 0:
                ot_sb[t] = ot_pool.tile([P, HID], BF16, name="ot", tag="ot")
                oth_sb[t] = oth_pool.tile([P, HID], F8, name="oth",
                                          tag="oth")
                otl_sb[t] = otl_pool.tile([P, HID], F8, name="otl",
                                          tag="otl")
            ot, oth, otl = ot_sb[t], oth_sb[t], otl_sb[t]
            hs = slice(hf * HW, (hf + 1) * HW)
            nc.sync.dma_start_transpose(
                ot[:, hs].rearrange("p (o t2) -> p o t2", t2=P),
                obf_sb[t][:, hs])
            # both split ops on Pool: ACT stays free of transpose-DMA
            # dependencies, so the PE's psum copies never queue behind them
            nc.gpsimd.tensor_copy(oth[:, hs], ot[:, hs])
            nc.gpsimd.tensor_sub(otl[:, hs], ot[:, hs], oth[:, hs])

        def transpose_split_o(t):
            split_half(t, 0)
            split_half(t, 1)
            obf_sb[t] = None
            ot_sb[t] = None

        def copy_qkv(which, c, ps, qsb, ksb, vsb):
            if which == "q":
                nc.scalar.copy(qsb[:, c * 512:(c + 1) * 512], ps[:])
            elif which == "k":
                nc.scalar.copy(ksb[:], ps[:])
            else:
                nc.scalar.mul(vsb[:], ps[:], 1.0 / WS)

        def emit_y_tile(tw):
            if tw == NTT - 1:
                # last tile: sequential chains, final ones narrow, so the
                # post-matmul drain holds only one short copy+DMA
                for c0, c1 in ((0, 512), (512, 1024), (1024, 1536),
                               (1536, 1792), (1792, 2048)):
                    yps = y_ps_pool.tile([P, c1 - c0], F32,
                                         name="yps", tag="yps")
                    emit_chain_pair([(yps, y_steps(tw, c0, c1))])
                    ysb = ysb_pool.tile([P, c1 - c0], BF16,
                                        name="ysb", tag="ysb")
                    nc.scalar.mul(ysb[:], yps[:], 1.0 / (WS * OS))
                    nc.sync.dma_start(
                        y[tw * P:(tw + 1) * P, c0:c1], ysb[:])
                oth_sb[tw] = None
                otl_sb[tw] = None
                return
            for sp in range(2):
                ypa = y_ps_pool.tile([P, 512], F32, name="yps", tag="yps")
                ypb = y_ps_pool.tile([P, 512], F32, name="yps", tag="yps")
                emit_chain_pair([
                    (ypa, y_steps(tw, 2 * sp * 512, (2 * sp + 1) * 512)),
                    (ypb, y_steps(tw, (2 * sp + 1) * 512, (2 * sp + 2) * 512)),
                ])
                for yps, s_ in ((ypa, 2 * sp), (ypb, 2 * sp + 1)):
                    ysb = ysb_pool.tile([P, 512], BF16, name="ysb", tag="ysb")
                    nc.scalar.mul(ysb[:], yps[:], 1.0 / (WS * OS))
                    nc.sync.dma_start(
                        y[tw * P:(tw + 1) * P, s_ * 512:(s_ + 1) * 512],
                        ysb[:])
            oth_sb[tw] = None
            otl_sb[tw] = None

        # ---- prologue: tiles 0-3 in three phases, each running 8 psum
        # chains (all banks) step-major so every weight chunk is consumed
        # by 4 reader tiles as it arrives ----
        NPT = 3
        pro_sb = {}
        for tt in range(NPT):
            pro_sb[tt] = (
                qsb_pool.tile([P, HID], BF16, tag="q", name="q"),
                qsb_pool.tile([P, KV], BF16, tag="k", name="k"),
                qsb_pool.tile([P, KV], BF16, tag="v", name="v"),
            )
        for chains in ([("q", 0), ("q", 1)], [("q", 2), ("q", 3)],
                       [("k", 0)], [("v", 0)]):
            pairs = []
            for ci, (which, c) in enumerate(chains):
                for tt in range(NPT):
                    pool = qkv_ps_pool if ci == 0 else y_ps_pool
                    tagname = "ps" if ci == 0 else "yps"
                    ps = pool.tile([P, 512], F32, name=tagname,
                                   tag=tagname)
                    pairs.append(
                        (ps, qkv_steps(tt, which, c),
                         (lambda w_, c_, t_: lambda psd: copy_qkv(
                             w_, c_, psd, *pro_sb[t_]))(which, c, tt)))
            emit_chain_pair(pairs)
        # attn + o-split interleaved per half-tile so each split sits
        # directly behind its weighted-sum in the Pool queue and the first
        # y chains start after only half an attn
        for tt in range(NPT):
            for hf in range(2):
                attn_half(tt, *pro_sb[tt], hf)
                split_half(tt, hf)
            obf_sb[tt] = None
            ot_sb[tt] = None
        load_xh(3)
        load_xl(3)
        load_xh(4)
        load_xl(4)

        # ---- steady state: QKV(t) | Wo(t-3 or t-4); the wo weight stream
        # is emitted at the first loop iteration, after every qkv weight and
        # the early x tiles, so the first y chains never wait on it ----
        for t in range(NPT, NTT + 4):
            if t <= NTT - 3:
                load_xh(t + 2)
                load_xl(t + 2)
            if t == NPT:
                # rest of wo, in Y(0)'s consumption order: its first chain
                # pair tails with wol cols 0:1024, then the second pair
                # opens on woh cols 1024:2048
                for j in range(4):
                    wchunk(wola, wol, 4 * j, 4 * (j + 1), 0, 1024, HID)
                for j in range(4):
                    wchunk(woha, woh, 4 * j, 4 * (j + 1), 1024, 2048, HID)
                for j in range(4):
                    wchunk(wola, wol, 4 * j, 4 * (j + 1), 1024, 2048, HID)

            if t < NTT:
                qsb = qsb_pool.tile([P, HID], BF16, tag="q")
                ksb = qsb_pool.tile([P, KV], BF16, tag="k")
                vsb = qsb_pool.tile([P, KV], BF16, tag="v")

                # 3 pair-interleaved chains; paired chains share the
                # stationary x pair per step (back-to-back reuse)
                for pi, (pa, pb) in enumerate(((("q", 0), ("k", 0)),
                                               (("q", 1), ("q", 2)),
                                               (("q", 3), ("v", 0)))):
                    pool = y_ps_pool if (t == NPT and pi == 2) \
                        else qkv_ps_pool
                    tg = "yps" if (t == NPT and pi == 2) else "ps"
                    psa = pool.tile([P, 512], F32, name=tg, tag=tg)
                    psb = pool.tile([P, 512], F32, name=tg, tag=tg)
                    emit_chain_pair([
                        (psa, qkv_steps(t, pa[0], pa[1])),
                        (psb, qkv_steps(t, pb[0], pb[1])),
                    ])
                    copy_qkv(pa[0], pa[1], psa, qsb, ksb, vsb)
                    copy_qkv(pb[0], pb[1], psb, qsb, ksb, vsb)

                attn_middle(t, qsb, ksb, vsb)

            # Wo matmuls + y out, lagged 5 tiles behind the QKV stream so
            # the first y chains never wait on the prologue's attn backlog
            # (4 tiles of DVE/Pool work drain at ~8us/iter) nor on the
            # wo weight DMA
            if t - 4 >= 0:
                emit_y_tile(t - 4)

            if NPT <= t < NTT:
                transpose_split_o(t)

    nc.compile()
    return nc


def _build_bias(has_bias: bool = True) -> bass.Bass:
    """Original (slower) path, kept for the biased case."""
    nc = bacc.Bacc("TRN2")
    x = nc.dram_tensor("x", [TPC, HID], BF16, kind="ExternalInput")
    wq = nc.dram_tensor("wq", [HID, HID], BF16, kind="ExternalInput")
    wk = nc.dram_tensor("wk", [HID, KV], BF16, kind="ExternalInput")
    wv = nc.dram_tensor("wv", [HID, KV], BF16, kind="ExternalInput")
    wo = nc.dram_tensor("wo", [HID, HID], BF16, kind="ExternalInput")
    if has_bias:
        bqkv = nc.dram_tensor("bqkv", [1, HID + 2 * KV], F32, kind="ExternalInput")
        bo = nc.dram_tensor("bo", [1, HID], F32, kind="ExternalInput")
    y = nc.dram_tensor("y", [TPC, HID], F32, kind="ExternalOutput")

    with tile.TileContext(nc) as tc, ExitStack() as ctx:
        const_pool = ctx.enter_context(tc.tile_pool(name="const", bufs=1))
        ident = const_pool.tile([P, P], BF16)
        make_identity(nc, ident[:])

        if has_bias:
            bias_qkv = const_pool.tile([P, HID + 2 * KV], F32)
            nc.sync.dma_start(bias_qkv[:], bqkv[0:1, :].broadcast_to((P, HID + 2 * KV)))
            bias_o = const_pool.tile([P, HID], F32)
            nc.sync.dma_start(bias_o[:], bo[0:1, :].broadcast_to((P, HID)))

        # O^T staging for the whole core: [o_block(16) x tokens(2048)] bf16
        ofm_pool = ctx.enter_context(tc.tile_pool(name="ofm", bufs=1))
        ofm = ofm_pool.tile([P, NI * TPC], BF16)

        kv_pool = ctx.enter_context(tc.tile_pool(name="wkv", bufs=1))
        wk_sb = []
        wv_sb = []
        for i in range(NI):
            wk_t = kv_pool.tile([P, KV], BF16, tag=f"wk{i}")
            nc.sync.dma_start(wk_t[:], wk[i * P:(i + 1) * P, :])
            wk_sb.append(wk_t)
            wv_t = kv_pool.tile([P, KV], BF16, tag=f"wv{i}")
            nc.sync.dma_start(wv_t[:], wv[i * P:(i + 1) * P, :])
            wv_sb.append(wv_t)

        pt_pool = ctx.enter_context(tc.tile_pool(name="pt", bufs=2, space="PSUM"))
        mm_pool = ctx.enter_context(tc.tile_pool(name="mm", bufs=3, space="PSUM"))

        # ---------------- Phase A: QKV projections + attention ----------------
        with tc.tile_pool(name="wqp", bufs=1) as wq_pool, \
             tc.tile_pool(name="xt", bufs=3) as xt_pool, \
             tc.tile_pool(name="xfm", bufs=1) as xfm_pool, \
             tc.tile_pool(name="qkv", bufs=1) as qkv_pool, \
             tc.tile_pool(name="attn", bufs=2) as attn_pool, \
             tc.tile_pool(name="oacc", bufs=1) as oacc_pool, \
             tc.tile_pool(name="obf", bufs=1) as obf_pool:
            wq_sb = []
            for i in range(NI):
                wq_t = wq_pool.tile([P, HID], BF16, tag=f"wq{i}")
                nc.sync.dma_start(wq_t[:], wq[i * P:(i + 1) * P, :])
                wq_sb.append(wq_t)

            for t in range(NTT):
                xt = xt_pool.tile([P, HID], BF16)
                nc.sync.dma_start(xt[:], x[t * P:(t + 1) * P, :])

                # transpose X tile to feature-major [i, t] (16 blocks of 128x128)
                xfm = xfm_pool.tile([P, HID], BF16)
                for j in range(4):
                    pt = pt_pool.tile([P, 512], BF16)
                    for k in range(4):
                        blk = 4 * j + k
                        nc.tensor.transpose(
                            pt[:, k * P:(k + 1) * P],
                            xt[:, blk * P:(blk + 1) * P],
                            ident[:],
                        )
                    nc.vector.tensor_copy(xfm[:, j * 512:(j + 1) * 512], pt[:])

                # QKV projections, token-major out: [t(128part), 3072]
                qkv = qkv_pool.tile([P, HID + 2 * KV], F32)
                for s in range(6):
                    ps = mm_pool.tile([P, 512], F32)
                    for i in range(NI):
                        if s < 4:
                            rhs = wq_sb[i][:, s * 512:(s + 1) * 512]
                        elif s == 4:
                            rhs = wk_sb[i][:]
                        else:
                            rhs = wv_sb[i][:]
                        nc.tensor.matmul(
                            ps[:], xfm[:, i * P:(i + 1) * P], rhs,
                            start=(i == 0), stop=(i == NI - 1),
                        )
                    if has_bias:
                        nc.vector.tensor_add(
                            qkv[:, s * 512:(s + 1) * 512], ps[:],
                            bias_qkv[:, s * 512:(s + 1) * 512],
                        )
                    else:
                        nc.vector.tensor_copy(qkv[:, s * 512:(s + 1) * 512], ps[:])

                # scores[t, h, g] = <q_h, k_g> * SCALE   (fused mult+reduce)
                sc = attn_pool.tile([P, H * G], F32, tag="sc")
                junk = attn_pool.tile([P, D], F32, tag="junk")
                for h in range(H):
                    for g in range(G):
                        nc.vector.scalar_tensor_tensor(
                            junk[:],
                            qkv[:, h * D:(h + 1) * D],
                            SCALE,
                            qkv[:, HID + g * D:HID + (g + 1) * D],
                            op0=mybir.AluOpType.mult,
                            op1=mybir.AluOpType.mult,
                            accum_out=sc[:, ds(h * G + g, 1)],
                        )

                # softmax over g (4); denominator folded into final scale
                ex = attn_pool.tile([P, H * G], F32, tag="ex")
                nc.scalar.activation(ex[:], sc[:], mybir.ActivationFunctionType.Exp)
                dn = attn_pool.tile([P, H], F32, tag="dn")
                nc.vector.reduce_sum(
                    dn[:], ex[:].rearrange("p (h g) -> p h g", g=G),
                    axis=mybir.AxisListType.X,
                )
                rc = attn_pool.tile([P, H], F32, tag="rc")
                nc.vector.reciprocal(rc[:], dn[:])

                # o[t, h*D+d] = (sum_g ex[t,h,g] * v[t, g*D+d]) * rc[t,h]
                acc = oacc_pool.tile([P, HID], F32, tag="acc")
                tmp = oacc_pool.tile([P, HID], F32, tag="tmp")
                obf = obf_pool.tile([P, HID], BF16)
                ab = [acc, tmp]
                for h in range(H):
                    hs = ds(h * D, D)
                    nc.vector.tensor_scalar_mul(
                        ab[0][:, hs],
                        qkv[:, HID + KV:HID + KV + D],
                        ex[:, ds(h * G, 1)],
                    )
                    for g in range(1, G):
                        nc.vector.scalar_tensor_tensor(
                            ab[g % 2][:, hs],
                            qkv[:, HID + KV + g * D:HID + KV + (g + 1) * D],
                            ex[:, ds(h * G + g, 1)],
                            ab[(g - 1) % 2][:, hs],
                            op0=mybir.AluOpType.mult,
                            op1=mybir.AluOpType.add,
                        )
                    nc.vector.tensor_scalar_mul(
                        obf[:, hs], ab[(G - 1) % 2][:, hs], rc[:, ds(h, 1)])

                # transpose O tile into ofm [o_block, token]
                for j in range(4):
                    pt = pt_pool.tile([P, 512], BF16)
                    for k in range(4):
                        blk = 4 * j + k
                        nc.tensor.transpose(
                            pt[:, k * P:(k + 1) * P],
                            obf[:, blk * P:(blk + 1) * P],
                            ident[:],
                        )
                    nc.vector.tensor_copy(
                        ofm[:].rearrange("p (o t) -> p o t", t=TPC)
                              [:, 4 * j:4 * j + 4, t * P:(t + 1) * P],
                        pt[:].rearrange("p (o t) -> p o t", t=P),
                    )

        # ---------------- Phase B: output projection ----------------
        with tc.tile_pool(name="wop", bufs=1) as wo_pool, \
             tc.tile_pool(name="yt", bufs=3) as yt_pool:
            wo_sb = []
            for i in range(NI):
                wo_t = wo_pool.tile([P, HID], BF16, tag=f"wo{i}")
                nc.sync.dma_start(wo_t[:], wo[i * P:(i + 1) * P, :])
                wo_sb.append(wo_t)

            for t in range(NTT):
                for s in range(4):
                    ps = mm_pool.tile([P, 512], F32)
                    for o in range(NI):
                        nc.tensor.matmul(
                            ps[:],
                            ofm[:, ds(o * TPC + t * P, P)],
                            wo_sb[o][:, s * 512:(s + 1) * 512],
                            start=(o == 0), stop=(o == NI - 1),
                        )
                    yt = yt_pool.tile([P, 512], F32)
                    if has_bias:
                        nc.vector.tensor_add(
                            yt[:], ps[:], bias_o[:, s * 512:(s + 1) * 512])
                    else:
                        nc.vector.tensor_copy(yt[:], ps[:])
                    nc.sync.dma_start(
                        y[t * P:(t + 1) * P, s * 512:(s + 1) * 512], yt[:])

    nc.compile()
    return nc


def _build(has_bias: bool) -> bass.Bass:
    return _build_bias(True) if has_bias else _build_fp8()


def kernel(hidden_states, Wq, bq, Wk, bk, Wv, bv, Wo, bo, _profile=None):
    has_bias = bool(np.any(bq) or np.any(bk) or np.any(bv) or np.any(bo))
    key = has_bias
    if key not in _cache:
        _cache[key] = _build(has_bias)
    nc = _cache[key]

    x_flat = np.ascontiguousarray(
        np.asarray(hidden_states, dtype=np.float32).reshape(NTOK, HID))

    in_maps = []
    if has_bias:
        bf = ml_dtypes.bfloat16
        xb = x_flat.astype(bf)
        wq_b = np.asarray(Wq, dtype=np.float32).astype(bf)
        wk_b = np.asarray(Wk, dtype=np.float32).astype(bf)
        wv_b = np.asarray(Wv, dtype=np.float32).astype(bf)
        wo_b = np.asarray(Wo, dtype=np.float32).astype(bf)
        for c in range(NCORES):
            m = {
                "x": np.ascontiguousarray(xb[c * TPC:(c + 1) * TPC]),
                "wq": wq_b, "wk": wk_b, "wv": wv_b, "wo": wo_b,
                "bqkv": np.concatenate([
                    np.asarray(bq, np.float32), np.asarray(bk, np.float32),
                    np.asarray(bv, np.float32)]).reshape(1, HID + 2 * KV),
                "bo": np.asarray(bo, np.float32).reshape(1, HID),
            }
            in_maps.append(m)
    else:
        e4 = ml_dtypes.float8_e4m3
        xh8 = x_flat.astype(e4)
        xl8 = (x_flat - xh8.astype(np.float32)).astype(e4)

        def wsplit(W):
            Wf = np.asarray(W, dtype=np.float32) * WS
            hi = Wf.astype(e4)
            lo = (Wf - hi.astype(np.float32)).astype(e4)
            return np.ascontiguousarray(hi), np.ascontiguousarray(lo)

        wq8 = np.ascontiguousarray(
            (np.asarray(Wq, np.float32) * WS).astype(e4))
        wk8 = np.ascontiguousarray(
            (np.asarray(Wk, np.float32) * WS).astype(e4))
        wvh8, wvl8 = wsplit(Wv)
        woh8, wol8 = wsplit(Wo)

        def pret(a):
            # host pre-transpose: row (t*128+p), col (i*128+tok) <- x[(t,tok),(i,p)]
            return np.ascontiguousarray(
                a.reshape(NTT, P, NI, P).transpose(0, 3, 2, 1).reshape(TPC, HID))

        for c in range(NCORES):
            m = {
                "xh": pret(xh8[c * TPC:(c + 1) * TPC]),
                "xl": pret(xl8[c * TPC:(c + 1) * TPC]),
                "wq": wq8, "wk": wk8,
                "wvh": wvh8, "wvl": wvl8,
                "woh": woh8, "wol": wol8,
            }
            in_maps.append(m)

    kwargs = dict(_profile) if _profile else {}
    kwargs.pop("result", None)
    res = run_bass_kernel_spmd(nc, in_maps, list(range(NCORES)), **kwargs)
    out = np.concatenate([r["y"] for r in res.results], axis=0)
    if _profile is not None:
        _profile["result"] = res
    return out.reshape(B, S, HID).astype(np.float32)
